# revision 27
# baseline (speedup 1.0000x reference)
"""Trainium2 Bass kernel for the LoE tiled-MLP (NeRF-style coordinate net).

Sharding: data-parallel over the pixel axis. N=262144 rows are split
contiguously across 8 cores (32768 rows each). Because the per-layer
expert tiles are contiguous row blocks, each core only ever needs a
contiguous slice of every weight tensor -> zero cross-core traffic.

Default build (_build3): all weights/activations fp16 (rel err ~8e-4,
budget 2e-2), chunks of 512 pixels processed in interleave groups of 3.
  - Positional encoding: coords pre-split on host into two f16 parts
    (xh=f16(x), xl'=f16((x-xh)*2^6)) so tps = smat5.T @ [xh,yh,xl',yl',1]
    is a single-pass f16 matmul reconstructing x to ~2^-24 rel; the
    smat5 entries are exact powers of two in f16. Then the magic-round
    r = t - round(t) on DVE and sin(2*pi*r) on ACT (table range +-pi).
  - The enc phase for group g+1 is emitted right after group g's L0 so
    the DVE/ACT enc chain clears before the PE needs it (no boundary
    stall), and mid-layer weight DMAs are emitted lazily a few groups
    ahead so the coords DMA is never queued behind megabytes of weights.
  - LeakyReLU(0.2) is a single HW Prelu op on ACT (the trig_and_small
    table holds sin+parametric_relu together, so one table load) for
    L0..L3, and a 2-op DVE form for L4 (PSUM->f16 copy + f16 max) to
    keep the ACT queue empty at the group boundary.
  - fp16 matmuls stream 1 row/cycle and keep the compiler's fast-weight-
    load path enabled (fp32-HIGH matmuls would disable it), so LDWEIGHTS
    hides under the previous matmul.
PSUM budget: 3 x [128,1024] layer tiles (6 banks) + 2 x 1-bank ring for
enc output / final [3,512] output = 8 banks exactly.
"""

import os
import sys

import numpy as np

sys.path.insert(0, "/opt/trn_rl_repo")

import concourse.bass as bass
import concourse.bacc as bacc
import concourse.mybir as mybir
import concourse.tile as tile
from concourse.alu_op_type import AluOpType
from concourse.bass_utils import run_bass_kernel_spmd

F32 = mybir.dt.float32
F32R = mybir.dt.float32r
BF16 = mybir.dt.bfloat16
F16 = mybir.dt.float16
ACT_SIN = mybir.ActivationFunctionType.Sin

N = 262144
NCORES = 8
ROWS = N // NCORES          # 32768 rows per core
CH = 512                    # pixels per chunk (psum free-dim, fp32 max)
K = 13                      # frequencies
H = 256
PE_SC = 2 * 2 * K + 2       # 52 sin/cos + 2 linearized coord rows
COORD_S = float(2.0 ** -11)  # tiny freq: sin(2*pi*s*c) ~ 2*pi*s*c, rel err 1.6e-6
MAGIC = float(1.5 * 2 ** 23)
TWO_PI = float(2.0 * np.pi)

# local (per-core) expert-tile row extents for layers 1..4
TILE_ROWS = {1: 65536, 2: 16384, 3: 4096, 4: 1024}

TRACE = False
LAST = {}


def _build(rows, f32r=True, stage_cols=2048, lrelu_eng=("a", "a", "a", "a", "a")):
    """Build the SPMD single-core Bass program for `rows` pixels."""
    nchunks = rows // CH
    stage_cols = min(stage_cols, rows)
    cpg = stage_cols // CH                       # chunks per DMA stage
    ntile = {l: max(rows // TILE_ROWS[l], 1) for l in (1, 2, 3, 4)}
    # chunk j -> local tile index for layer l
    tidx = {l: [min(j * CH // TILE_ROWS[l], ntile[l] - 1) for j in range(nchunks)]
            for l in (1, 2, 3, 4)}

    MDT = F32R if f32r else F32
    nc = bacc.Bacc()
    d_coords = nc.dram_tensor("coordsT3", [3, rows], F32, kind="ExternalInput")
    d_smat = nc.dram_tensor("smat", [3, PE_SC], F32, kind="ExternalInput")
    d_w0s = nc.dram_tensor("w0s", [PE_SC, H], MDT, kind="ExternalInput")
    d_wmid = {l: nc.dram_tensor(f"w{l}", [ntile[l], H, H], MDT, kind="ExternalInput")
              for l in (1, 2, 3, 4)}
    d_wl = nc.dram_tensor("wlT", [H, 3], MDT, kind="ExternalInput")
    d_out = nc.dram_tensor("out", [3, rows], F32, kind="ExternalOutput")

    def mdt(ap):
        return ap

    def lrelu(mode, xt, ps, rt):
        """xt(sbuf) = LeakyReLU_0.2(ps).  rt: scratch sbuf tile.

        Only ACT and DVE can read PSUM, and at most one tensor operand of a
        DVE op may live in PSUM, hence the two-pass forms.
        """
        if mode == "a":      # ACT relu + DVE combine
            nc.scalar.activation(rt[:], ps[:], mybir.ActivationFunctionType.Relu,
                                 scale=0.8)
            nc.vector.scalar_tensor_tensor(xt[:], ps[:], 0.2, rt[:],
                                           AluOpType.mult, AluOpType.add)
        elif mode == "v":    # DVE relu + DVE combine
            nc.vector.tensor_scalar(rt[:], ps[:], 0.0, 0.8,
                                    AluOpType.max, AluOpType.mult)
            nc.vector.scalar_tensor_tensor(xt[:], ps[:], 0.2, rt[:],
                                           AluOpType.mult, AluOpType.add)
        elif mode == "hwl":  # single ACT op, HW Lrelu table (alpha slope)
            nc.scalar.activation(xt[:], ps[:], mybir.ActivationFunctionType.Lrelu,
                                 alpha=0.2)
        elif mode == "hwp":  # single ACT op, HW Prelu table (alpha slope)
            nc.scalar.activation(xt[:], ps[:], mybir.ActivationFunctionType.Prelu,
                                 alpha=0.2)
        else:
            raise ValueError(mode)

    with tile.TileContext(nc) as tc:
        with (
            tc.tile_pool(name="wp", bufs=1) as wp,
            tc.tile_pool(name="io", bufs=2) as iop,
            tc.tile_pool(name="ac", bufs=2) as acp,
            tc.tile_pool(name="psa", bufs=2, space="PSUM") as ppa,
            tc.tile_pool(name="psb", bufs=3, space="PSUM") as ppb,
        ):
            # ---- resident weights (DMA once, first-use order) ----
            smat_sb = wp.tile([3, PE_SC], F32, tag="smat")
            nc.sync.dma_start(out=smat_sb[:], in_=d_smat[:])
            w0s_sb = wp.tile([PE_SC, H], MDT, tag="w0s")
            nc.sync.dma_start(out=w0s_sb[:], in_=d_w0s[:])
            wl_sb = []
            for kb in range(2):
                t = wp.tile([128, 3], MDT, tag=f"wl{kb}")
                nc.sync.dma_start(out=t[:], in_=d_wl[kb * 128:(kb + 1) * 128, :])
                wl_sb.append(t)

            wmid_sb = {l: [[None, None] for _ in range(ntile[l])] for l in (1, 2, 3, 4)}
            order = []
            for l in (1, 2, 3, 4):
                for t in range(ntile[l]):
                    first = min(j for j in range(nchunks) if tidx[l][j] == t)
                    order.append((first, l, t))
            order.sort()
            for _, l, t in order:
                for kb in range(2):
                    w = wp.tile([128, H], MDT, tag=f"w{l}_{t}_{kb}")
                    nc.sync.dma_start(
                        out=w[:], in_=d_wmid[l][t, kb * 128:(kb + 1) * 128, :])
                    wmid_sb[l][t][kb] = w

            # ---- main chunk loop: pairs of chunks, layer-interleaved ----
            # PE is an in-order queue: emitting chunk j+1's matmuls right
            # after chunk j's same-layer matmuls means every PE wait (on the
            # LeakyReLU chain) has independent work queued behind it.
            cr = None
            ot = None
            st = {}
            for jj in range(0, nchunks, 2):
                pair = [j for j in (jj, jj + 1) if j < nchunks]
                for j in pair:
                    g, o = divmod(j, cpg)
                    if o == 0:
                        cr = iop.tile([3, stage_cols], F32, tag="cr")
                        nc.sync.dma_start(
                            out=cr[:],
                            in_=d_coords[:, g * stage_cols:(g + 1) * stage_cols])
                        ot = iop.tile([3, stage_cols], F32, tag="ot")
                    rc = cr[:, o * CH:(o + 1) * CH]      # [3, 512] coords+ones
                    tps = ppa.tile([PE_SC, CH], F32, tag="ang")
                    nc.tensor.matmul(tps[:], smat_sb[:], rc, start=True, stop=True)
                    st[j] = {"rc": rc, "tps": tps, "ot": ot, "g": g, "o": o}
                for j in pair:
                    s = st[j]
                    rnd = acp.tile([PE_SC, CH], F32, tag="rnd")
                    nc.vector.tensor_scalar(rnd[:], s["tps"][:], MAGIC, MAGIC,
                                            AluOpType.add, AluOpType.subtract)
                    frac = acp.tile([PE_SC, CH], F32, tag="frac")
                    nc.vector.tensor_tensor(frac[:], s["tps"][:], rnd[:],
                                            AluOpType.subtract)
                    sc = acp.tile([PE_SC, CH], MDT, tag="sc")
                    nc.scalar.activation(sc[:], frac[:], ACT_SIN, scale=TWO_PI)
                    s["sc"] = sc
                for j in pair:
                    s = st[j]
                    ps = ppb.tile([128, 2 * CH], F32, tag="lps")
                    for ob in range(2):
                        nc.tensor.matmul(ps[:, ob * CH:(ob + 1) * CH],
                                         w0s_sb[:, ob * 128:(ob + 1) * 128],
                                         s["sc"][:], start=True, stop=True)
                    x = acp.tile([128, 2 * CH], MDT, tag="x0")
                    rt = acp.tile([128, 2 * CH], F32, tag="rt")
                    lrelu(lrelu_eng[0], x, ps, rt)
                    s["x"] = x
                for l in (1, 2, 3, 4):
                    for j in pair:
                        s = st[j]
                        wt = wmid_sb[l][tidx[l][j]]
                        ps = ppb.tile([128, 2 * CH], F32, tag="lps")
                        for ob in range(2):
                            osl = slice(ob * CH, (ob + 1) * CH)
                            wsl = slice(ob * 128, (ob + 1) * 128)
                            for kb in range(2):
                                nc.tensor.matmul(
                                    ps[:, osl], wt[kb][:, wsl],
                                    s["x"][:, kb * CH:(kb + 1) * CH],
                                    start=(kb == 0), stop=(kb == 1))
                        xn = acp.tile([128, 2 * CH], MDT, tag=f"x{l}")
                        rt = acp.tile([128, 2 * CH], F32, tag="rt")
                        lrelu(lrelu_eng[l], xn, ps, rt)
                        s["x"] = xn
                for j in pair:
                    s = st[j]
                    po = ppb.tile([3, CH], F32, tag="lps")
                    for kb in range(2):
                        nc.tensor.matmul(po[:], wl_sb[kb][:],
                                         s["x"][:, kb * CH:(kb + 1) * CH],
                                         start=(kb == 0), stop=(kb == 1))
                    nc.scalar.copy(s["ot"][:, s["o"] * CH:(s["o"] + 1) * CH], po[:])
                    if s["o"] == cpg - 1:
                        nc.sync.dma_start(
                            out=d_out[:, s["g"] * stage_cols:(s["g"] + 1) * stage_cols],
                            in_=s["ot"][:])
                    del st[j]
    nc.finalize()
    return nc


def _build2(rows, G=3, lrelu_eng=("hwp", "hwp", "v", "hwp", "hwp"),
            stage_cols=2048, f32r=True, bf16=False, f16=False, lazy_w=True,
            enc_mod=False, prefetch_groups=3, copy_eng="v"):
    """Restructured SPMD program: G-chunk interleave groups.

    Per group of G chunks, each layer emits the G chunks' matmuls
    back-to-back so the PE always has >=2 chunks of independent work
    queued behind every LeakyReLU-chain wait. LeakyReLU is a single
    ACT-engine Prelu op ("hwp") for most layers; one layer runs on DVE
    ("v", two ops) to balance engine load. Output copy PSUM->SBUF is on
    DVE. PSUM budget: lps ring 3x2 banks + ang/po ring 2x1 bank = 8.

    bf16: weights+activations in bf16 (enc path stays fp32). Halves
    LDWEIGHTS time and weight DMA.
    lazy_w: emit mid-layer weight DMAs just-in-time (prefetch_groups
    ahead) so the stage-0 coords DMA isn't queued behind 11MB of
    weights on the in-order sync queue.
    enc_mod: one-op range reduction frac2 = (t mod 1) - 0.5 on DVE and
    sin(-2*pi*frac2) = sin(2*pi*t) on ACT, instead of the two-op
    magic-round form.
    """
    nchunks = rows // CH
    stage_cols = min(stage_cols, rows)
    cpg = stage_cols // CH
    ntile = {l: max(rows // TILE_ROWS[l], 1) for l in (1, 2, 3, 4)}
    tidx = {l: [min(j * CH // TILE_ROWS[l], ntile[l] - 1) for j in range(nchunks)]
            for l in (1, 2, 3, 4)}

    MDT = F16 if f16 else (BF16 if bf16 else (F32R if f32r else F32))
    nc = bacc.Bacc()
    d_coords = nc.dram_tensor("coordsT3", [3, rows], F32, kind="ExternalInput")
    d_smat = nc.dram_tensor("smat", [3, PE_SC], F32, kind="ExternalInput")
    d_w0s = nc.dram_tensor("w0s", [PE_SC, H], MDT, kind="ExternalInput")
    d_wmid = {l: nc.dram_tensor(f"w{l}", [ntile[l], H, H], MDT, kind="ExternalInput")
              for l in (1, 2, 3, 4)}
    d_wl = nc.dram_tensor("wlT", [H, 3], MDT, kind="ExternalInput")
    d_out = nc.dram_tensor("out", [3, rows], F32, kind="ExternalOutput")

    def lrelu(mode, xt, ps, scratch_pool):
        if mode == "hwp":     # single ACT op, HW Prelu table (alpha slope)
            nc.scalar.activation(xt[:], ps[:], mybir.ActivationFunctionType.Prelu,
                                 alpha=0.2)
        elif mode == "a":     # ACT relu + DVE combine
            rt = scratch_pool.tile(list(xt.shape), F32, tag="rt")
            nc.scalar.activation(rt[:], ps[:], mybir.ActivationFunctionType.Relu,
                                 scale=0.8)
            nc.vector.scalar_tensor_tensor(xt[:], ps[:], 0.2, rt[:],
                                           AluOpType.mult, AluOpType.add)
        elif mode == "v":     # DVE-only: relu+scale then combine
            rt = scratch_pool.tile(list(xt.shape), F32, tag="rt")
            nc.vector.tensor_scalar(rt[:], ps[:], 0.0, 0.8,
                                    AluOpType.max, AluOpType.mult)
            nc.vector.scalar_tensor_tensor(xt[:], ps[:], 0.2, rt[:],
                                           AluOpType.mult, AluOpType.add)
        else:
            raise ValueError(mode)

    with tile.TileContext(nc) as tc:
        with (
            tc.tile_pool(name="wp", bufs=1) as wp,
            tc.tile_pool(name="io", bufs=2) as iop,
            tc.tile_pool(name="ac", bufs=3) as acp,
            tc.tile_pool(name="rt", bufs=2) as rtp,
            tc.tile_pool(name="psa", bufs=2, space="PSUM") as ppa,
            tc.tile_pool(name="psb", bufs=3, space="PSUM") as ppb,
        ):
            # ---- small resident weights up front ----
            smat_sb = wp.tile([3, PE_SC], F32, tag="smat")
            nc.sync.dma_start(out=smat_sb[:], in_=d_smat[:])
            w0s_sb = wp.tile([PE_SC, H], MDT, tag="w0s")
            nc.sync.dma_start(out=w0s_sb[:], in_=d_w0s[:])
            wl_sb = []
            for kb in range(2):
                t = wp.tile([128, 3], MDT, tag=f"wl{kb}")
                nc.sync.dma_start(out=t[:], in_=d_wl[kb * 128:(kb + 1) * 128, :])
                wl_sb.append(t)

            # ---- mid-layer weights: eager (first-use order) or lazy ----
            wmid_sb = {l: [None] * ntile[l] for l in (1, 2, 3, 4)}

            def ensure_w(l, t):
                if wmid_sb[l][t] is None:
                    blk = []
                    for kb in range(2):
                        w = wp.tile([128, H], MDT, tag=f"w{l}_{t}_{kb}")
                        nc.sync.dma_start(
                            out=w[:], in_=d_wmid[l][t, kb * 128:(kb + 1) * 128, :])
                        blk.append(w)
                    wmid_sb[l][t] = blk

            if not lazy_w:
                order = []
                for l in (1, 2, 3, 4):
                    for t in range(ntile[l]):
                        first = min(j for j in range(nchunks) if tidx[l][j] == t)
                        order.append((first, l, t))
                order.sort()
                for _, l, t in order:
                    ensure_w(l, t)

            # ---- main loop: groups of G chunks, layer-lockstep ----
            cr = None
            ot = None
            st = {}
            for gg in range(0, nchunks, G):
                group = list(range(gg, min(gg + G, nchunks)))
                for j in group:
                    g, o = divmod(j, cpg)
                    if o == 0:
                        cr = iop.tile([3, stage_cols], F32, tag="cr")
                        nc.sync.dma_start(
                            out=cr[:],
                            in_=d_coords[:, g * stage_cols:(g + 1) * stage_cols])
                        ot = iop.tile([3, stage_cols], F32, tag="ot")
                    rc = cr[:, o * CH:(o + 1) * CH]
                    tps = ppa.tile([PE_SC, CH], F32, tag="ang")
                    nc.tensor.matmul(tps[:], smat_sb[:], rc, start=True, stop=True)
                    st[j] = {"tps": tps, "ot": ot, "g": g, "o": o}
                if lazy_w:
                    lim = min(gg + G * prefetch_groups, nchunks)
                    for l in (1, 2, 3, 4):
                        for j in range(gg, lim):
                            ensure_w(l, tidx[l][j])
                for j in group:
                    s = st[j]
                    if enc_mod:
                        frac = acp.tile([PE_SC, CH], F32, tag="frac")
                        nc.vector.tensor_scalar(frac[:], s["tps"][:], 1.0, 0.5,
                                                AluOpType.mod, AluOpType.subtract)
                        sc = acp.tile([PE_SC, CH], MDT, tag="sc")
                        nc.scalar.activation(sc[:], frac[:], ACT_SIN, scale=-TWO_PI)
                    else:
                        rnd = acp.tile([PE_SC, CH], F32, tag="rnd")
                        nc.vector.tensor_scalar(rnd[:], s["tps"][:], MAGIC, MAGIC,
                                                AluOpType.add, AluOpType.subtract)
                        frac = acp.tile([PE_SC, CH], F32, tag="frac")
                        nc.vector.tensor_tensor(frac[:], s["tps"][:], rnd[:],
                                                AluOpType.subtract)
                        sc = acp.tile([PE_SC, CH], MDT, tag="sc")
                        nc.scalar.activation(sc[:], frac[:], ACT_SIN, scale=TWO_PI)
                    s["sc"] = sc
                for pos, j in enumerate(group):
                    s = st[j]
                    ps = ppb.tile([128, 2 * CH], F32, tag="lps")
                    for ob in range(2):
                        nc.tensor.matmul(ps[:, ob * CH:(ob + 1) * CH],
                                         w0s_sb[:, ob * 128:(ob + 1) * 128],
                                         s["sc"][:], start=True, stop=True)
                    x = acp.tile([128, 2 * CH], MDT, tag="x0")
                    lrelu(lmode(0, pos), x, ps, rtp)
                    s["x"] = x
                for l in (1, 2, 3, 4):
                    for j in group:
                        s = st[j]
                        wt = wmid_sb[l][tidx[l][j]]
                        ps = ppb.tile([128, 2 * CH], F32, tag="lps")
                        for ob in range(2):
                            osl = slice(ob * CH, (ob + 1) * CH)
                            wsl = slice(ob * 128, (ob + 1) * 128)
                            for kb in range(2):
                                nc.tensor.matmul(
                                    ps[:, osl], wt[kb][:, wsl],
                                    s["x"][:, kb * CH:(kb + 1) * CH],
                                    start=(kb == 0), stop=(kb == 1))
                        xn = acp.tile([128, 2 * CH], MDT, tag=f"x{l}")
                        lrelu(lrelu_eng[l], xn, ps, rtp)
                        s["x"] = xn
                for j in group:
                    s = st[j]
                    po = ppa.tile([3, CH], F32, tag="ang")
                    for kb in range(2):
                        nc.tensor.matmul(po[:], wl_sb[kb][:],
                                         s["x"][:, kb * CH:(kb + 1) * CH],
                                         start=(kb == 0), stop=(kb == 1))
                    if copy_eng == "v":
                        nc.vector.tensor_scalar(
                            s["ot"][:, s["o"] * CH:(s["o"] + 1) * CH], po[:],
                            0.0, None, AluOpType.add)
                    else:
                        nc.scalar.copy(
                            s["ot"][:, s["o"] * CH:(s["o"] + 1) * CH], po[:])
                    if s["o"] == cpg - 1:
                        nc.sync.dma_start(
                            out=d_out[:, s["g"] * stage_cols:(s["g"] + 1) * stage_cols],
                            in_=s["ot"][:])
                    del st[j]
    nc.finalize()
    return nc


def _build3(rows, G=3,
            lrelu_eng=("hwp", "hwp", ("hwp", "hwp", "v2h"), "hwp",
                       ("v2", "v2", "hwp")),
            stage_cols=2048, copy_eng="v", prefetch_groups=4, rotate=False):
    """f16 build with single-pass f16 positional-encoding matmul and a
    software-pipelined encoding stage.

    Coordinates are pre-split on host into two f16 parts
    (xh = f16(x), xl' = f16((x - xh) * 2^6)) so the enc matmul
      tps = smat5.T @ [xh, yh, xl', yl', 1]
    is pure f16 (1 cycle/row, keeps FWL enabled) while reconstructing
    x to ~2^-24 relative: smat5 rows hold 2^(k-1) and 2^(k-1-6), all
    exact powers of two in f16. The enc phase for group g+1 is emitted
    right after group g's L0 so the rnd/frac/sin chain clears long
    before the PE needs sc, killing the group-boundary stall.
    """
    nchunks = rows // CH
    stage_cols = min(stage_cols, rows)
    cpg = stage_cols // CH
    ntile = {l: max(rows // TILE_ROWS[l], 1) for l in (1, 2, 3, 4)}
    tidx = {l: [min(j * CH // TILE_ROWS[l], ntile[l] - 1) for j in range(nchunks)]
            for l in (1, 2, 3, 4)}

    MDT = F16
    nc = bacc.Bacc()
    d_coords = nc.dram_tensor("coords5", [5, rows], F16, kind="ExternalInput")
    d_smat = nc.dram_tensor("smat5", [5, PE_SC], F16, kind="ExternalInput")
    d_w0s = nc.dram_tensor("w0s", [PE_SC, H], MDT, kind="ExternalInput")
    d_wmid = {l: nc.dram_tensor(f"w{l}", [ntile[l], H, H], MDT, kind="ExternalInput")
              for l in (1, 2, 3, 4)}
    d_wl = nc.dram_tensor("wlT", [H, 3], MDT, kind="ExternalInput")
    d_out = nc.dram_tensor("out", [3, rows], F32, kind="ExternalOutput")

    def lrelu(mode, xt, ps, rtp):
        if mode == "hwp":
            nc.scalar.activation(xt[:], ps[:], mybir.ActivationFunctionType.Prelu,
                                 alpha=0.2)
        elif mode == "a":
            rt = rtp.tile(list(xt.shape), F32, tag="rt")
            nc.scalar.activation(rt[:], ps[:], mybir.ActivationFunctionType.Relu,
                                 scale=0.8)
            nc.vector.scalar_tensor_tensor(xt[:], ps[:], 0.2, rt[:],
                                           AluOpType.mult, AluOpType.add)
        elif mode == "v":
            rt = rtp.tile(list(xt.shape), F32, tag="rt")
            nc.vector.tensor_scalar(rt[:], ps[:], 0.0, 0.8,
                                    AluOpType.max, AluOpType.mult)
            nc.vector.scalar_tensor_tensor(xt[:], ps[:], 0.2, rt[:],
                                           AluOpType.mult, AluOpType.add)
        elif mode == "hwp2":  # Prelu in two halves: first half ready sooner
            h = xt.shape[-1] // 2
            nc.scalar.activation(xt[:, :h], ps[:, :h],
                                 mybir.ActivationFunctionType.Prelu, alpha=0.2)
            nc.scalar.activation(xt[:, h:], ps[:, h:],
                                 mybir.ActivationFunctionType.Prelu, alpha=0.2)
        elif mode == "v2":    # DVE: PSUM->f16 copy, then all-SBUF f16 max
            yv = rtp.tile(list(xt.shape), F16, tag="rt16")
            nc.vector.tensor_scalar(yv[:], ps[:], 0.0, None, AluOpType.add)
            nc.vector.scalar_tensor_tensor(xt[:], yv[:], 0.2, yv[:],
                                           AluOpType.mult, AluOpType.max)
        elif mode == "v2h":   # v2 in column halves: first half ready sooner,
            h = xt.shape[-1] // 2  # so the next layer's kb0 matmuls can start
            yv = rtp.tile(list(xt.shape), F16, tag="rt16")
            for sl in (slice(0, h), slice(h, 2 * h)):
                nc.vector.tensor_scalar(yv[:, sl], ps[:, sl], 0.0, None,
                                        AluOpType.add)
                nc.vector.scalar_tensor_tensor(xt[:, sl], yv[:, sl], 0.2,
                                               yv[:, sl], AluOpType.mult,
                                               AluOpType.max)
        else:
            raise ValueError(mode)

    with tile.TileContext(nc) as tc:
        with (
            tc.tile_pool(name="wp", bufs=1) as wp,
            tc.tile_pool(name="io", bufs=2) as iop,
            tc.tile_pool(name="ac", bufs=3) as acp,
            tc.tile_pool(name="rt", bufs=2) as rtp,
            tc.tile_pool(name="psa", bufs=2, space="PSUM") as ppa,
            tc.tile_pool(name="psb", bufs=3, space="PSUM") as ppb,
        ):
            smat_sb = wp.tile([5, PE_SC], F16, tag="smat")
            nc.sync.dma_start(out=smat_sb[:], in_=d_smat[:])

            wmid_sb = {l: [None] * ntile[l] for l in (1, 2, 3, 4)}

            def ensure_w(l, t):
                if wmid_sb[l][t] is None:
                    blk = []
                    for kb in range(2):
                        w = wp.tile([128, H], MDT, tag=f"w{l}_{t}_{kb}")
                        nc.sync.dma_start(
                            out=w[:], in_=d_wmid[l][t, kb * 128:(kb + 1) * 128, :])
                        blk.append(w)
                    wmid_sb[l][t] = blk

            st = {}
            io_state = {"cr": None, "ot": None}

            def emit_enc(group):
                """Enc phase for `group`: matmul + rnd/frac (DVE) + sin (ACT)."""
                for j in group:
                    g, o = divmod(j, cpg)
                    if o == 0:
                        cr_t = iop.tile([5, stage_cols], F16, tag="cr")
                        nc.sync.dma_start(
                            out=cr_t[:],
                            in_=d_coords[:, g * stage_cols:(g + 1) * stage_cols])
                        ot_t = iop.tile([3, stage_cols], F32, tag="ot")
                        io_state["cr"], io_state["ot"] = cr_t, ot_t
                    rc = io_state["cr"][:, o * CH:(o + 1) * CH]
                    tps = ppa.tile([PE_SC, CH], F32, tag="ang")
                    nc.tensor.matmul(tps[:], smat_sb[:], rc, start=True, stop=True)
                    st[j] = {"tps": tps, "ot": io_state["ot"], "g": g, "o": o}
                for j in group:
                    s = st[j]
                    rnd = acp.tile([PE_SC, CH], F32, tag="rnd")
                    nc.vector.tensor_scalar(rnd[:], s["tps"][:], MAGIC, MAGIC,
                                            AluOpType.add, AluOpType.subtract)
                    frac = acp.tile([PE_SC, CH], F32, tag="frac")
                    nc.vector.tensor_tensor(frac[:], s["tps"][:], rnd[:],
                                            AluOpType.subtract)
                    sc = acp.tile([PE_SC, CH], MDT, tag="sc")
                    nc.scalar.activation(sc[:], frac[:], ACT_SIN, scale=TWO_PI)
                    s["sc"] = sc

            def lmode(l, pos):
                e = lrelu_eng[l]
                return e if isinstance(e, str) else e[min(pos, len(e) - 1)]

            groups = [list(range(gg, min(gg + G, nchunks)))
                      for gg in range(0, nchunks, G)]
            if len(groups) > 1 and len(groups[-1]) < G:
                groups[-2].extend(groups.pop())
            emit_enc(groups[0])
            w0s_sb = wp.tile([PE_SC, H], MDT, tag="w0s")
            nc.sync.dma_start(out=w0s_sb[:], in_=d_w0s[:])
            wl_sb = []
            for kb in range(2):
                t = wp.tile([128, 3], MDT, tag=f"wl{kb}")
                nc.sync.dma_start(out=t[:], in_=d_wl[kb * 128:(kb + 1) * 128, :])
                wl_sb.append(t)
            for l in (1, 2, 3, 4):
                for j in range(min(G * (prefetch_groups + 5), nchunks)):
                    ensure_w(l, tidx[l][j])
            for gi, group in enumerate(groups):
                # L0
                for pos, j in enumerate(group):
                    s = st[j]
                    ps = ppb.tile([128, 2 * CH], F32, tag="lps")
                    for ob in range(2):
                        nc.tensor.matmul(ps[:, ob * CH:(ob + 1) * CH],
                                         w0s_sb[:, ob * 128:(ob + 1) * 128],
                                         s["sc"][:], start=True, stop=True)
                    x = acp.tile([128, 2 * CH], MDT, tag="x0")
                    lrelu(lmode(0, pos), x, ps, rtp)
                    s["x"] = x
                # L1..L4; the enc phase for group g+1 is emitted after L2 so
                # the DVE is free during L1/L2 (mid-layer v2 units run promptly)
                # while sin(g+1) still clears an entire L3+L4+last ahead of its
                # consumer
                for l in (1, 2, 3, 4):
                    if l == 3 and gi + 1 < len(groups):
                        emit_enc(groups[gi + 1])
                        lim = min(groups[gi + 1][-1] + 1 + G * (prefetch_groups - 1),
                                  nchunks)
                        for lw in (1, 2, 3, 4):
                            for j in range(groups[gi + 1][0], lim):
                                ensure_w(lw, tidx[lw][j])
                    lorder = (group[1:] + group[:1]) if (rotate and l >= 3) \
                        else group
                    for pos, j in enumerate(lorder):
                        s = st[j]
                        wt = wmid_sb[l][tidx[l][j]]
                        ps = ppb.tile([128, 2 * CH], F32, tag="lps")
                        for ob in range(2):
                            osl = slice(ob * CH, (ob + 1) * CH)
                            wsl = slice(ob * 128, (ob + 1) * 128)
                            for kb in range(2):
                                nc.tensor.matmul(
                                    ps[:, osl], wt[kb][:, wsl],
                                    s["x"][:, kb * CH:(kb + 1) * CH],
                                    start=(kb == 0), stop=(kb == 1))
                        xn = acp.tile([128, 2 * CH], MDT, tag=f"x{l}")
                        lrelu(lmode(l, pos), xn, ps, rtp)
                        s["x"] = xn
                # last layer + output copy
                for pos, j in enumerate((group[1:] + group[:1]) if rotate
                                        else group):
                    s = st[j]
                    po = ppa.tile([3, CH], F32, tag="ang")
                    for kb in range(2):
                        nc.tensor.matmul(po[:], wl_sb[kb][:],
                                         s["x"][:, kb * CH:(kb + 1) * CH],
                                         start=(kb == 0), stop=(kb == 1))
                    if copy_eng == "v":
                        nc.vector.tensor_scalar(
                            s["ot"][:, s["o"] * CH:(s["o"] + 1) * CH], po[:],
                            0.0, None, AluOpType.add)
                    else:
                        nc.scalar.copy(
                            s["ot"][:, s["o"] * CH:(s["o"] + 1) * CH], po[:])
                    if s["o"] == cpg - 1:
                        nc.sync.dma_start(
                            out=d_out[:, s["g"] * stage_cols:(s["g"] + 1) * stage_cols],
                            in_=s["ot"][:])
                    del st[j]
    nc.finalize()
    return nc


def _host_prep(coords, w0, w1, w2, w3, w4, w_last, rows, bf16=False, f16=False,
               enc5=False):
    """Split full inputs into per-core in_maps."""
    if f16:
        wdt = np.float16
    elif bf16:
        import ml_dtypes
        wdt = ml_dtypes.bfloat16
    else:
        wdt = np.float32
    coords = np.asarray(coords, np.float32)
    if enc5:
        smat = np.zeros((5, PE_SC), np.float16)
        for p in range(PE_SC - 2):
            k, f, s = p >> 2, (p >> 1) & 1, p & 1
            smat[f, p] = np.float16(2.0 ** (k - 1))
            smat[2 + f, p] = np.float16(2.0 ** (k - 1 - 6))
            smat[4, p] = 0.25 if s else 0.0
        smat[0, PE_SC - 2] = np.float16(COORD_S)
        smat[1, PE_SC - 1] = np.float16(COORD_S)
    else:
        smat = np.zeros((3, PE_SC), np.float32)
        for p in range(PE_SC - 2):
            k, f, s = p >> 2, (p >> 1) & 1, p & 1
            smat[f, p] = float(2.0 ** (k - 1))
            smat[2, p] = 0.25 if s else 0.0
        smat[0, PE_SC - 2] = COORD_S
        smat[1, PE_SC - 1] = COORD_S
    w0 = np.asarray(w0, np.float32)[0]              # [54, 256]
    w0s = np.empty((PE_SC, H), np.float32)
    w0s[:PE_SC - 2] = w0[2:]
    w0s[PE_SC - 2:] = w0[0:2] / np.float32(2.0 * np.pi * COORD_S)
    w0s = w0s.astype(wdt)
    wlT = np.ascontiguousarray(np.asarray(w_last, np.float32).T).astype(wdt)
    wmid_full = {1: np.asarray(w1, np.float32).astype(wdt),
                 2: np.asarray(w2, np.float32).astype(wdt),
                 3: np.asarray(w3, np.float32).astype(wdt),
                 4: np.asarray(w4, np.float32).astype(wdt)}
    ntile = {l: max(rows // TILE_ROWS[l], 1) for l in (1, 2, 3, 4)}
    in_maps = []
    for c in range(NCORES):
        sl = coords[c * rows:(c + 1) * rows]
        if enc5:
            chT = sl.T.astype(np.float16)                    # [2, rows] high
            clT = ((sl.T - chT.astype(np.float32)) * 64.0).astype(np.float16)
            c5 = np.empty((5, rows), np.float16)
            c5[0:2] = chT
            c5[2:4] = clT
            c5[4] = 1.0
            m = {"coords5": c5, "smat5": smat, "w0s": w0s, "wlT": wlT}
        else:
            ct3 = np.empty((3, rows), np.float32)
            ct3[0:2] = sl.T
            ct3[2] = 1.0
            m = {"coordsT3": ct3, "smat": smat, "w0s": w0s, "wlT": wlT}
        for l in (1, 2, 3, 4):
            w = wmid_full[l]
            t0 = c * rows // (N // w.shape[0]) if w.shape[0] * rows >= N else 0
            t0 = (c * rows) // (N // w.shape[0])
            m[f"w{l}"] = np.ascontiguousarray(w[t0:t0 + ntile[l]])
        in_maps.append(m)
    return in_maps


_BUILT = {}


def kernel(coords, w0, b0, w1, b1, w2, b2, w3, b3, w4, b4, w_last, b_last,
           f32r=True, lrelu_eng=None, variant="v3", G=3, bf16=False,
           f16=False, lazy_w=True, enc_mod=False, rotate=False):
    if variant == "v3":
        if rotate:
            le = lrelu_eng or ("hwp", "hwp", ("v2", "hwp", "hwp"), "hwp",
                              ("v2", "v2", "hwp"))
        else:
            le = lrelu_eng or ("hwp", "hwp", ("hwp", "hwp", "v2h"), "hwp",
                              ("v2", "v2", "hwp"))
        key = ("v3", ROWS, repr(le), G, rotate)
        if key not in _BUILT:
            _BUILT[key] = _build3(ROWS, G=G, lrelu_eng=le, rotate=rotate)
        nc = _BUILT[key]
        in_maps = _host_prep(coords, w0, w1, w2, w3, w4, w_last, ROWS,
                             f16=True, enc5=True)
        res = run_bass_kernel_spmd(nc, in_maps, list(range(NCORES)), trace=TRACE)
        LAST["res"] = res
        out = np.empty((N, 3), np.float32)
        for c in range(NCORES):
            out[c * ROWS:(c + 1) * ROWS, :] = res.results[c]["out"].T
        return out
    if variant == "g3":
        le = lrelu_eng or ("hwp", "hwp", "v", "hwp", "hwp")
        key = ("g3", ROWS, bool(f32r), tuple(le), G, bf16, f16, lazy_w, enc_mod)
        if key not in _BUILT:
            _BUILT[key] = _build2(ROWS, G=G, f32r=f32r, lrelu_eng=le, bf16=bf16,
                                  f16=f16, lazy_w=lazy_w, enc_mod=enc_mod)
    else:
        bf16 = False
        f16 = False
        le = lrelu_eng or ("a", "a", "a", "a", "a")
        key = (ROWS, bool(f32r), tuple(le))
        if key not in _BUILT:
            _BUILT[key] = _build(ROWS, f32r=f32r, lrelu_eng=le)
    nc = _BUILT[key]
    in_maps = _host_prep(coords, w0, w1, w2, w3, w4, w_last, ROWS, bf16=bf16, f16=f16)
    res = run_bass_kernel_spmd(nc, in_maps, list(range(NCORES)), trace=TRACE)
    LAST["res"] = res
    out = np.empty((N, 3), np.float32)
    for c in range(NCORES):
        out[c * ROWS:(c + 1) * ROWS, :] = res.results[c]["out"].T
    return out



# revision 28
# speedup vs baseline: 1.0825x; 1.0825x over previous
"""Trainium2 Bass kernel for the LoE tiled-MLP (NeRF-style coordinate net).

Sharding: data-parallel over the pixel axis. N=262144 rows are split
contiguously across 8 cores (32768 rows each). Because the per-layer
expert tiles are contiguous row blocks, each core only ever needs a
contiguous slice of every weight tensor -> zero cross-core traffic.

Default build (_build3): all weights/activations fp16 (rel err ~8e-4,
budget 2e-2), chunks of 512 pixels processed in interleave groups of 3.
  - Positional encoding: coords pre-split on host into two f16 parts
    (xh=f16(x), xl'=f16((x-xh)*2^6)) so tps = smat5.T @ [xh,yh,xl',yl',1]
    is a single-pass f16 matmul reconstructing x to ~2^-24 rel; the
    smat5 entries are exact powers of two in f16. Then the magic-round
    r = t - round(t) on DVE and sin(2*pi*r) on ACT (table range +-pi).
  - The enc phase for group g+1 is emitted right after group g's L0 so
    the DVE/ACT enc chain clears before the PE needs it (no boundary
    stall), and mid-layer weight DMAs are emitted lazily a few groups
    ahead so the coords DMA is never queued behind megabytes of weights.
  - LeakyReLU(0.2) is a single HW Prelu op on ACT (the trig_and_small
    table holds sin+parametric_relu together, so one table load) for
    L0..L3, and a 2-op DVE form for L4 (PSUM->f16 copy + f16 max) to
    keep the ACT queue empty at the group boundary.
  - fp16 matmuls stream 1 row/cycle and keep the compiler's fast-weight-
    load path enabled (fp32-HIGH matmuls would disable it), so LDWEIGHTS
    hides under the previous matmul.
PSUM budget: 3 x [128,1024] layer tiles (6 banks) + 2 x 1-bank ring for
enc output / final [3,512] output = 8 banks exactly.
"""

import os
import sys

import numpy as np

sys.path.insert(0, "/opt/trn_rl_repo")

import concourse.bass as bass
import concourse.bacc as bacc
import concourse.mybir as mybir
import concourse.tile as tile
from concourse.alu_op_type import AluOpType
from concourse.bass_utils import run_bass_kernel_spmd

F32 = mybir.dt.float32
F32R = mybir.dt.float32r
BF16 = mybir.dt.bfloat16
F16 = mybir.dt.float16
ACT_SIN = mybir.ActivationFunctionType.Sin

N = 262144
NCORES = 8
ROWS = N // NCORES          # 32768 rows per core
CH = 512                    # pixels per chunk (psum free-dim, fp32 max)
K = 13                      # frequencies
H = 256
PE_SC = 2 * 2 * K + 2       # 52 sin/cos + 2 linearized coord rows
COORD_S = float(2.0 ** -11)  # tiny freq: sin(2*pi*s*c) ~ 2*pi*s*c, rel err 1.6e-6
MAGIC = float(1.5 * 2 ** 23)
TWO_PI = float(2.0 * np.pi)

# local (per-core) expert-tile row extents for layers 1..4
TILE_ROWS = {1: 65536, 2: 16384, 3: 4096, 4: 1024}

TRACE = False
LAST = {}


def _build(rows, f32r=True, stage_cols=2048, lrelu_eng=("a", "a", "a", "a", "a")):
    """Build the SPMD single-core Bass program for `rows` pixels."""
    nchunks = rows // CH
    stage_cols = min(stage_cols, rows)
    cpg = stage_cols // CH                       # chunks per DMA stage
    ntile = {l: max(rows // TILE_ROWS[l], 1) for l in (1, 2, 3, 4)}
    # chunk j -> local tile index for layer l
    tidx = {l: [min(j * CH // TILE_ROWS[l], ntile[l] - 1) for j in range(nchunks)]
            for l in (1, 2, 3, 4)}

    MDT = F32R if f32r else F32
    nc = bacc.Bacc()
    d_coords = nc.dram_tensor("coordsT3", [3, rows], F32, kind="ExternalInput")
    d_smat = nc.dram_tensor("smat", [3, PE_SC], F32, kind="ExternalInput")
    d_w0s = nc.dram_tensor("w0s", [PE_SC, H], MDT, kind="ExternalInput")
    d_wmid = {l: nc.dram_tensor(f"w{l}", [ntile[l], H, H], MDT, kind="ExternalInput")
              for l in (1, 2, 3, 4)}
    d_wl = nc.dram_tensor("wlT", [H, 3], MDT, kind="ExternalInput")
    d_out = nc.dram_tensor("out", [3, rows], F32, kind="ExternalOutput")

    def mdt(ap):
        return ap

    def lrelu(mode, xt, ps, rt):
        """xt(sbuf) = LeakyReLU_0.2(ps).  rt: scratch sbuf tile.

        Only ACT and DVE can read PSUM, and at most one tensor operand of a
        DVE op may live in PSUM, hence the two-pass forms.
        """
        if mode == "a":      # ACT relu + DVE combine
            nc.scalar.activation(rt[:], ps[:], mybir.ActivationFunctionType.Relu,
                                 scale=0.8)
            nc.vector.scalar_tensor_tensor(xt[:], ps[:], 0.2, rt[:],
                                           AluOpType.mult, AluOpType.add)
        elif mode == "v":    # DVE relu + DVE combine
            nc.vector.tensor_scalar(rt[:], ps[:], 0.0, 0.8,
                                    AluOpType.max, AluOpType.mult)
            nc.vector.scalar_tensor_tensor(xt[:], ps[:], 0.2, rt[:],
                                           AluOpType.mult, AluOpType.add)
        elif mode == "hwl":  # single ACT op, HW Lrelu table (alpha slope)
            nc.scalar.activation(xt[:], ps[:], mybir.ActivationFunctionType.Lrelu,
                                 alpha=0.2)
        elif mode == "hwp":  # single ACT op, HW Prelu table (alpha slope)
            nc.scalar.activation(xt[:], ps[:], mybir.ActivationFunctionType.Prelu,
                                 alpha=0.2)
        else:
            raise ValueError(mode)

    with tile.TileContext(nc) as tc:
        with (
            tc.tile_pool(name="wp", bufs=1) as wp,
            tc.tile_pool(name="io", bufs=2) as iop,
            tc.tile_pool(name="ac", bufs=2) as acp,
            tc.tile_pool(name="psa", bufs=2, space="PSUM") as ppa,
            tc.tile_pool(name="psb", bufs=3, space="PSUM") as ppb,
        ):
            # ---- resident weights (DMA once, first-use order) ----
            smat_sb = wp.tile([3, PE_SC], F32, tag="smat")
            nc.sync.dma_start(out=smat_sb[:], in_=d_smat[:])
            w0s_sb = wp.tile([PE_SC, H], MDT, tag="w0s")
            nc.sync.dma_start(out=w0s_sb[:], in_=d_w0s[:])
            wl_sb = []
            for kb in range(2):
                t = wp.tile([128, 3], MDT, tag=f"wl{kb}")
                nc.sync.dma_start(out=t[:], in_=d_wl[kb * 128:(kb + 1) * 128, :])
                wl_sb.append(t)

            wmid_sb = {l: [[None, None] for _ in range(ntile[l])] for l in (1, 2, 3, 4)}
            order = []
            for l in (1, 2, 3, 4):
                for t in range(ntile[l]):
                    first = min(j for j in range(nchunks) if tidx[l][j] == t)
                    order.append((first, l, t))
            order.sort()
            for _, l, t in order:
                for kb in range(2):
                    w = wp.tile([128, H], MDT, tag=f"w{l}_{t}_{kb}")
                    nc.sync.dma_start(
                        out=w[:], in_=d_wmid[l][t, kb * 128:(kb + 1) * 128, :])
                    wmid_sb[l][t][kb] = w

            # ---- main chunk loop: pairs of chunks, layer-interleaved ----
            # PE is an in-order queue: emitting chunk j+1's matmuls right
            # after chunk j's same-layer matmuls means every PE wait (on the
            # LeakyReLU chain) has independent work queued behind it.
            cr = None
            ot = None
            st = {}
            for jj in range(0, nchunks, 2):
                pair = [j for j in (jj, jj + 1) if j < nchunks]
                for j in pair:
                    g, o = divmod(j, cpg)
                    if o == 0:
                        cr = iop.tile([3, stage_cols], F32, tag="cr")
                        nc.sync.dma_start(
                            out=cr[:],
                            in_=d_coords[:, g * stage_cols:(g + 1) * stage_cols])
                        ot = iop.tile([3, stage_cols], F32, tag="ot")
                    rc = cr[:, o * CH:(o + 1) * CH]      # [3, 512] coords+ones
                    tps = ppa.tile([PE_SC, CH], F32, tag="ang")
                    nc.tensor.matmul(tps[:], smat_sb[:], rc, start=True, stop=True)
                    st[j] = {"rc": rc, "tps": tps, "ot": ot, "g": g, "o": o}
                for j in pair:
                    s = st[j]
                    rnd = acp.tile([PE_SC, CH], F32, tag="rnd")
                    nc.vector.tensor_scalar(rnd[:], s["tps"][:], MAGIC, MAGIC,
                                            AluOpType.add, AluOpType.subtract)
                    frac = acp.tile([PE_SC, CH], F32, tag="frac")
                    nc.vector.tensor_tensor(frac[:], s["tps"][:], rnd[:],
                                            AluOpType.subtract)
                    sc = acp.tile([PE_SC, CH], MDT, tag="sc")
                    nc.scalar.activation(sc[:], frac[:], ACT_SIN, scale=TWO_PI)
                    s["sc"] = sc
                for j in pair:
                    s = st[j]
                    ps = ppb.tile([128, 2 * CH], F32, tag="lps")
                    for ob in range(2):
                        nc.tensor.matmul(ps[:, ob * CH:(ob + 1) * CH],
                                         w0s_sb[:, ob * 128:(ob + 1) * 128],
                                         s["sc"][:], start=True, stop=True)
                    x = acp.tile([128, 2 * CH], MDT, tag="x0")
                    rt = acp.tile([128, 2 * CH], F32, tag="rt")
                    lrelu(lrelu_eng[0], x, ps, rt)
                    s["x"] = x
                for l in (1, 2, 3, 4):
                    for j in pair:
                        s = st[j]
                        wt = wmid_sb[l][tidx[l][j]]
                        ps = ppb.tile([128, 2 * CH], F32, tag="lps")
                        for ob in range(2):
                            osl = slice(ob * CH, (ob + 1) * CH)
                            wsl = slice(ob * 128, (ob + 1) * 128)
                            for kb in range(2):
                                nc.tensor.matmul(
                                    ps[:, osl], wt[kb][:, wsl],
                                    s["x"][:, kb * CH:(kb + 1) * CH],
                                    start=(kb == 0), stop=(kb == 1))
                        xn = acp.tile([128, 2 * CH], MDT, tag=f"x{l}")
                        rt = acp.tile([128, 2 * CH], F32, tag="rt")
                        lrelu(lrelu_eng[l], xn, ps, rt)
                        s["x"] = xn
                for j in pair:
                    s = st[j]
                    po = ppb.tile([3, CH], F32, tag="lps")
                    for kb in range(2):
                        nc.tensor.matmul(po[:], wl_sb[kb][:],
                                         s["x"][:, kb * CH:(kb + 1) * CH],
                                         start=(kb == 0), stop=(kb == 1))
                    nc.scalar.copy(s["ot"][:, s["o"] * CH:(s["o"] + 1) * CH], po[:])
                    if s["o"] == cpg - 1:
                        nc.sync.dma_start(
                            out=d_out[:, s["g"] * stage_cols:(s["g"] + 1) * stage_cols],
                            in_=s["ot"][:])
                    del st[j]
    nc.finalize()
    return nc


def _build2(rows, G=3, lrelu_eng=("hwp", "hwp", "v", "hwp", "hwp"),
            stage_cols=2048, f32r=True, bf16=False, f16=False, lazy_w=True,
            enc_mod=False, prefetch_groups=3, copy_eng="v"):
    """Restructured SPMD program: G-chunk interleave groups.

    Per group of G chunks, each layer emits the G chunks' matmuls
    back-to-back so the PE always has >=2 chunks of independent work
    queued behind every LeakyReLU-chain wait. LeakyReLU is a single
    ACT-engine Prelu op ("hwp") for most layers; one layer runs on DVE
    ("v", two ops) to balance engine load. Output copy PSUM->SBUF is on
    DVE. PSUM budget: lps ring 3x2 banks + ang/po ring 2x1 bank = 8.

    bf16: weights+activations in bf16 (enc path stays fp32). Halves
    LDWEIGHTS time and weight DMA.
    lazy_w: emit mid-layer weight DMAs just-in-time (prefetch_groups
    ahead) so the stage-0 coords DMA isn't queued behind 11MB of
    weights on the in-order sync queue.
    enc_mod: one-op range reduction frac2 = (t mod 1) - 0.5 on DVE and
    sin(-2*pi*frac2) = sin(2*pi*t) on ACT, instead of the two-op
    magic-round form.
    """
    nchunks = rows // CH
    stage_cols = min(stage_cols, rows)
    cpg = stage_cols // CH
    ntile = {l: max(rows // TILE_ROWS[l], 1) for l in (1, 2, 3, 4)}
    tidx = {l: [min(j * CH // TILE_ROWS[l], ntile[l] - 1) for j in range(nchunks)]
            for l in (1, 2, 3, 4)}

    MDT = F16 if f16 else (BF16 if bf16 else (F32R if f32r else F32))
    nc = bacc.Bacc()
    d_coords = nc.dram_tensor("coordsT3", [3, rows], F32, kind="ExternalInput")
    d_smat = nc.dram_tensor("smat", [3, PE_SC], F32, kind="ExternalInput")
    d_w0s = nc.dram_tensor("w0s", [PE_SC, H], MDT, kind="ExternalInput")
    d_wmid = {l: nc.dram_tensor(f"w{l}", [ntile[l], H, H], MDT, kind="ExternalInput")
              for l in (1, 2, 3, 4)}
    d_wl = nc.dram_tensor("wlT", [H, 3], MDT, kind="ExternalInput")
    d_out = nc.dram_tensor("out", [3, rows], F32, kind="ExternalOutput")

    def lrelu(mode, xt, ps, scratch_pool):
        if mode == "hwp":     # single ACT op, HW Prelu table (alpha slope)
            nc.scalar.activation(xt[:], ps[:], mybir.ActivationFunctionType.Prelu,
                                 alpha=0.2)
        elif mode == "a":     # ACT relu + DVE combine
            rt = scratch_pool.tile(list(xt.shape), F32, tag="rt")
            nc.scalar.activation(rt[:], ps[:], mybir.ActivationFunctionType.Relu,
                                 scale=0.8)
            nc.vector.scalar_tensor_tensor(xt[:], ps[:], 0.2, rt[:],
                                           AluOpType.mult, AluOpType.add)
        elif mode == "v":     # DVE-only: relu+scale then combine
            rt = scratch_pool.tile(list(xt.shape), F32, tag="rt")
            nc.vector.tensor_scalar(rt[:], ps[:], 0.0, 0.8,
                                    AluOpType.max, AluOpType.mult)
            nc.vector.scalar_tensor_tensor(xt[:], ps[:], 0.2, rt[:],
                                           AluOpType.mult, AluOpType.add)
        else:
            raise ValueError(mode)

    with tile.TileContext(nc) as tc:
        with (
            tc.tile_pool(name="wp", bufs=1) as wp,
            tc.tile_pool(name="io", bufs=2) as iop,
            tc.tile_pool(name="ac", bufs=3) as acp,
            tc.tile_pool(name="rt", bufs=2) as rtp,
            tc.tile_pool(name="psa", bufs=2, space="PSUM") as ppa,
            tc.tile_pool(name="psb", bufs=3, space="PSUM") as ppb,
        ):
            # ---- small resident weights up front ----
            smat_sb = wp.tile([3, PE_SC], F32, tag="smat")
            nc.sync.dma_start(out=smat_sb[:], in_=d_smat[:])
            w0s_sb = wp.tile([PE_SC, H], MDT, tag="w0s")
            nc.sync.dma_start(out=w0s_sb[:], in_=d_w0s[:])
            wl_sb = []
            for kb in range(2):
                t = wp.tile([128, 3], MDT, tag=f"wl{kb}")
                nc.sync.dma_start(out=t[:], in_=d_wl[kb * 128:(kb + 1) * 128, :])
                wl_sb.append(t)

            # ---- mid-layer weights: eager (first-use order) or lazy ----
            wmid_sb = {l: [None] * ntile[l] for l in (1, 2, 3, 4)}

            def ensure_w(l, t):
                if wmid_sb[l][t] is None:
                    blk = []
                    for kb in range(2):
                        w = wp.tile([128, H], MDT, tag=f"w{l}_{t}_{kb}")
                        nc.sync.dma_start(
                            out=w[:], in_=d_wmid[l][t, kb * 128:(kb + 1) * 128, :])
                        blk.append(w)
                    wmid_sb[l][t] = blk

            if not lazy_w:
                order = []
                for l in (1, 2, 3, 4):
                    for t in range(ntile[l]):
                        first = min(j for j in range(nchunks) if tidx[l][j] == t)
                        order.append((first, l, t))
                order.sort()
                for _, l, t in order:
                    ensure_w(l, t)

            # ---- main loop: groups of G chunks, layer-lockstep ----
            cr = None
            ot = None
            st = {}
            for gg in range(0, nchunks, G):
                group = list(range(gg, min(gg + G, nchunks)))
                for j in group:
                    g, o = divmod(j, cpg)
                    if o == 0:
                        cr = iop.tile([3, stage_cols], F32, tag="cr")
                        nc.sync.dma_start(
                            out=cr[:],
                            in_=d_coords[:, g * stage_cols:(g + 1) * stage_cols])
                        ot = iop.tile([3, stage_cols], F32, tag="ot")
                    rc = cr[:, o * CH:(o + 1) * CH]
                    tps = ppa.tile([PE_SC, CH], F32, tag="ang")
                    nc.tensor.matmul(tps[:], smat_sb[:], rc, start=True, stop=True)
                    st[j] = {"tps": tps, "ot": ot, "g": g, "o": o}
                if lazy_w:
                    lim = min(gg + G * prefetch_groups, nchunks)
                    for l in (1, 2, 3, 4):
                        for j in range(gg, lim):
                            ensure_w(l, tidx[l][j])
                for j in group:
                    s = st[j]
                    if enc_mod:
                        frac = acp.tile([PE_SC, CH], F32, tag="frac")
                        nc.vector.tensor_scalar(frac[:], s["tps"][:], 1.0, 0.5,
                                                AluOpType.mod, AluOpType.subtract)
                        sc = acp.tile([PE_SC, CH], MDT, tag="sc")
                        nc.scalar.activation(sc[:], frac[:], ACT_SIN, scale=-TWO_PI)
                    else:
                        rnd = acp.tile([PE_SC, CH], F32, tag="rnd")
                        nc.vector.tensor_scalar(rnd[:], s["tps"][:], MAGIC, MAGIC,
                                                AluOpType.add, AluOpType.subtract)
                        frac = acp.tile([PE_SC, CH], F32, tag="frac")
                        nc.vector.tensor_tensor(frac[:], s["tps"][:], rnd[:],
                                                AluOpType.subtract)
                        sc = acp.tile([PE_SC, CH], MDT, tag="sc")
                        nc.scalar.activation(sc[:], frac[:], ACT_SIN, scale=TWO_PI)
                    s["sc"] = sc
                for pos, j in enumerate(group):
                    s = st[j]
                    ps = ppb.tile([128, 2 * CH], F32, tag="lps")
                    for ob in range(2):
                        nc.tensor.matmul(ps[:, ob * CH:(ob + 1) * CH],
                                         w0s_sb[:, ob * 128:(ob + 1) * 128],
                                         s["sc"][:], start=True, stop=True)
                    x = acp.tile([128, 2 * CH], MDT, tag="x0")
                    lrelu(lmode(0, pos), x, ps, rtp)
                    s["x"] = x
                for l in (1, 2, 3, 4):
                    for j in group:
                        s = st[j]
                        wt = wmid_sb[l][tidx[l][j]]
                        ps = ppb.tile([128, 2 * CH], F32, tag="lps")
                        for ob in range(2):
                            osl = slice(ob * CH, (ob + 1) * CH)
                            wsl = slice(ob * 128, (ob + 1) * 128)
                            for kb in range(2):
                                nc.tensor.matmul(
                                    ps[:, osl], wt[kb][:, wsl],
                                    s["x"][:, kb * CH:(kb + 1) * CH],
                                    start=(kb == 0), stop=(kb == 1))
                        xn = acp.tile([128, 2 * CH], MDT, tag=f"x{l}")
                        lrelu(lrelu_eng[l], xn, ps, rtp)
                        s["x"] = xn
                for j in group:
                    s = st[j]
                    po = ppa.tile([3, CH], F32, tag="ang")
                    for kb in range(2):
                        nc.tensor.matmul(po[:], wl_sb[kb][:],
                                         s["x"][:, kb * CH:(kb + 1) * CH],
                                         start=(kb == 0), stop=(kb == 1))
                    if copy_eng == "v":
                        nc.vector.tensor_scalar(
                            s["ot"][:, s["o"] * CH:(s["o"] + 1) * CH], po[:],
                            0.0, None, AluOpType.add)
                    else:
                        nc.scalar.copy(
                            s["ot"][:, s["o"] * CH:(s["o"] + 1) * CH], po[:])
                    if s["o"] == cpg - 1:
                        nc.sync.dma_start(
                            out=d_out[:, s["g"] * stage_cols:(s["g"] + 1) * stage_cols],
                            in_=s["ot"][:])
                    del st[j]
    nc.finalize()
    return nc


def _build3(rows, G=3,
            lrelu_eng=("hwp", "hwp", ("hwp", "hwp", "v2"), "hwp",
                       ("v2", "v2", "hwp")),
            stage_cols=2048, copy_eng="v", prefetch_groups=3, rotate=False):
    """f16 build with single-pass f16 positional-encoding matmul and a
    software-pipelined encoding stage.

    Coordinates are pre-split on host into two f16 parts
    (xh = f16(x), xl' = f16((x - xh) * 2^6)) so the enc matmul
      tps = smat5.T @ [xh, yh, xl', yl', 1]
    is pure f16 (1 cycle/row, keeps FWL enabled) while reconstructing
    x to ~2^-24 relative: smat5 rows hold 2^(k-1) and 2^(k-1-6), all
    exact powers of two in f16. The enc phase for group g+1 is emitted
    right after group g's L0 so the rnd/frac/sin chain clears long
    before the PE needs sc, killing the group-boundary stall.
    """
    nchunks = rows // CH
    stage_cols = min(stage_cols, rows)
    cpg = stage_cols // CH
    ntile = {l: max(rows // TILE_ROWS[l], 1) for l in (1, 2, 3, 4)}
    tidx = {l: [min(j * CH // TILE_ROWS[l], ntile[l] - 1) for j in range(nchunks)]
            for l in (1, 2, 3, 4)}

    MDT = F16
    nc = bacc.Bacc()
    d_coords = nc.dram_tensor("coords5", [5, rows], F16, kind="ExternalInput")
    d_smat = nc.dram_tensor("smat5", [5, PE_SC], F16, kind="ExternalInput")
    d_w0s = nc.dram_tensor("w0s", [PE_SC, H], MDT, kind="ExternalInput")
    d_wmid = {l: nc.dram_tensor(f"w{l}", [ntile[l], H, H], MDT, kind="ExternalInput")
              for l in (1, 2, 3, 4)}
    d_wl = nc.dram_tensor("wlT", [H, 3], MDT, kind="ExternalInput")
    d_out = nc.dram_tensor("out", [3, rows], F32, kind="ExternalOutput")

    def lrelu(mode, xt, ps, rtp):
        if mode == "hwp":
            nc.scalar.activation(xt[:], ps[:], mybir.ActivationFunctionType.Prelu,
                                 alpha=0.2)
        elif mode == "a":
            rt = rtp.tile(list(xt.shape), F32, tag="rt")
            nc.scalar.activation(rt[:], ps[:], mybir.ActivationFunctionType.Relu,
                                 scale=0.8)
            nc.vector.scalar_tensor_tensor(xt[:], ps[:], 0.2, rt[:],
                                           AluOpType.mult, AluOpType.add)
        elif mode == "v":
            rt = rtp.tile(list(xt.shape), F32, tag="rt")
            nc.vector.tensor_scalar(rt[:], ps[:], 0.0, 0.8,
                                    AluOpType.max, AluOpType.mult)
            nc.vector.scalar_tensor_tensor(xt[:], ps[:], 0.2, rt[:],
                                           AluOpType.mult, AluOpType.add)
        elif mode == "hwp2":  # Prelu in two halves: first half ready sooner
            h = xt.shape[-1] // 2
            nc.scalar.activation(xt[:, :h], ps[:, :h],
                                 mybir.ActivationFunctionType.Prelu, alpha=0.2)
            nc.scalar.activation(xt[:, h:], ps[:, h:],
                                 mybir.ActivationFunctionType.Prelu, alpha=0.2)
        elif mode == "v2":    # DVE: PSUM->f16 copy, then all-SBUF f16 max
            yv = rtp.tile(list(xt.shape), F16, tag="rt16")
            nc.vector.tensor_scalar(yv[:], ps[:], 0.0, None, AluOpType.add)
            nc.vector.scalar_tensor_tensor(xt[:], yv[:], 0.2, yv[:],
                                           AluOpType.mult, AluOpType.max)
        elif mode == "v2h":   # v2 in column halves: first half ready sooner,
            h = xt.shape[-1] // 2  # so the next layer's kb0 matmuls can start
            yv = rtp.tile(list(xt.shape), F16, tag="rt16")
            for sl in (slice(0, h), slice(h, 2 * h)):
                nc.vector.tensor_scalar(yv[:, sl], ps[:, sl], 0.0, None,
                                        AluOpType.add)
                nc.vector.scalar_tensor_tensor(xt[:, sl], yv[:, sl], 0.2,
                                               yv[:, sl], AluOpType.mult,
                                               AluOpType.max)
        else:
            raise ValueError(mode)

    with tile.TileContext(nc) as tc:
        with (
            tc.tile_pool(name="wp", bufs=1) as wp,
            tc.tile_pool(name="io", bufs=2) as iop,
            tc.tile_pool(name="ac", bufs=3) as acp,
            tc.tile_pool(name="rt", bufs=2) as rtp,
            tc.tile_pool(name="psa", bufs=2, space="PSUM") as ppa,
            tc.tile_pool(name="psb", bufs=3, space="PSUM") as ppb,
        ):
            smat_sb = wp.tile([5, PE_SC], F16, tag="smat")
            nc.sync.dma_start(out=smat_sb[:], in_=d_smat[:])

            wmid_sb = {l: [None] * ntile[l] for l in (1, 2, 3, 4)}

            def ensure_w(l, t):
                if wmid_sb[l][t] is None:
                    blk = []
                    for kb in range(2):
                        w = wp.tile([128, H], MDT, tag=f"w{l}_{t}_{kb}")
                        nc.sync.dma_start(
                            out=w[:], in_=d_wmid[l][t, kb * 128:(kb + 1) * 128, :])
                        blk.append(w)
                    wmid_sb[l][t] = blk

            st = {}
            io_state = {"cr": None, "ot": None}

            def emit_enc(group):
                """Enc phase for `group`: matmul + rnd/frac (DVE) + sin (ACT)."""
                for j in group:
                    g, o = divmod(j, cpg)
                    if o == 0:
                        cr_t = iop.tile([5, stage_cols], F16, tag="cr")
                        nc.sync.dma_start(
                            out=cr_t[:],
                            in_=d_coords[:, g * stage_cols:(g + 1) * stage_cols])
                        ot_t = iop.tile([3, stage_cols], F32, tag="ot")
                        io_state["cr"], io_state["ot"] = cr_t, ot_t
                    rc = io_state["cr"][:, o * CH:(o + 1) * CH]
                    tps = ppa.tile([PE_SC, CH], F32, tag="ang")
                    nc.tensor.matmul(tps[:], smat_sb[:], rc, start=True, stop=True)
                    st[j] = {"tps": tps, "ot": io_state["ot"], "g": g, "o": o}
                for j in group:
                    s = st[j]
                    rnd = acp.tile([PE_SC, CH], F32, tag="rnd")
                    nc.vector.tensor_scalar(rnd[:], s["tps"][:], MAGIC, MAGIC,
                                            AluOpType.add, AluOpType.subtract)
                    frac = acp.tile([PE_SC, CH], F32, tag="frac")
                    nc.vector.tensor_tensor(frac[:], s["tps"][:], rnd[:],
                                            AluOpType.subtract)
                    sc = acp.tile([PE_SC, CH], MDT, tag="sc")
                    nc.scalar.activation(sc[:], frac[:], ACT_SIN, scale=TWO_PI)
                    s["sc"] = sc

            def lmode(l, pos):
                e = lrelu_eng[l]
                return e if isinstance(e, str) else e[min(pos, len(e) - 1)]

            groups = [list(range(gg, min(gg + G, nchunks)))
                      for gg in range(0, nchunks, G)]
            if len(groups) > 1 and len(groups[-1]) < G:
                groups[-2].extend(groups.pop())
            emit_enc(groups[0])
            w0s_sb = wp.tile([PE_SC, H], MDT, tag="w0s")
            nc.sync.dma_start(out=w0s_sb[:], in_=d_w0s[:])
            wl_sb = []
            for kb in range(2):
                t = wp.tile([128, 3], MDT, tag=f"wl{kb}")
                nc.sync.dma_start(out=t[:], in_=d_wl[kb * 128:(kb + 1) * 128, :])
                wl_sb.append(t)
            for l in (1, 2, 3, 4):
                for j in range(min(G * (prefetch_groups + 5), nchunks)):
                    ensure_w(l, tidx[l][j])
            for gi, group in enumerate(groups):
                # L0
                for pos, j in enumerate(group):
                    s = st[j]
                    ps = ppb.tile([128, 2 * CH], F32, tag="lps")
                    for ob in range(2):
                        nc.tensor.matmul(ps[:, ob * CH:(ob + 1) * CH],
                                         w0s_sb[:, ob * 128:(ob + 1) * 128],
                                         s["sc"][:], start=True, stop=True)
                    x = acp.tile([128, 2 * CH], MDT, tag="x0")
                    lrelu(lmode(0, pos), x, ps, rtp)
                    s["x"] = x
                # L1..L4; the enc phase for group g+1 is emitted after L2 so
                # the DVE is free during L1/L2 (mid-layer v2 units run promptly)
                # while sin(g+1) still clears an entire L3+L4+last ahead of its
                # consumer
                for l in (1, 2, 3, 4):
                    if l == 3 and gi + 1 < len(groups):
                        emit_enc(groups[gi + 1])
                        lim = min(groups[gi + 1][-1] + 1 + G * (prefetch_groups - 1),
                                  nchunks)
                        for lw in (1, 2, 3, 4):
                            for j in range(groups[gi + 1][0], lim):
                                ensure_w(lw, tidx[lw][j])
                    lorder = (group[1:] + group[:1]) if (rotate and l >= 3) \
                        else group
                    for pos, j in enumerate(lorder):
                        s = st[j]
                        wt = wmid_sb[l][tidx[l][j]]
                        ps = ppb.tile([128, 2 * CH], F32, tag="lps")
                        for ob in range(2):
                            osl = slice(ob * CH, (ob + 1) * CH)
                            wsl = slice(ob * 128, (ob + 1) * 128)
                            for kb in range(2):
                                nc.tensor.matmul(
                                    ps[:, osl], wt[kb][:, wsl],
                                    s["x"][:, kb * CH:(kb + 1) * CH],
                                    start=(kb == 0), stop=(kb == 1))
                        xn = acp.tile([128, 2 * CH], MDT, tag=f"x{l}")
                        lrelu(lmode(l, pos), xn, ps, rtp)
                        s["x"] = xn
                # last layer + output copy
                for pos, j in enumerate((group[1:] + group[:1]) if rotate
                                        else group):
                    s = st[j]
                    po = ppa.tile([3, CH], F32, tag="ang")
                    for kb in range(2):
                        nc.tensor.matmul(po[:], wl_sb[kb][:],
                                         s["x"][:, kb * CH:(kb + 1) * CH],
                                         start=(kb == 0), stop=(kb == 1))
                    if copy_eng == "v":
                        nc.vector.tensor_scalar(
                            s["ot"][:, s["o"] * CH:(s["o"] + 1) * CH], po[:],
                            0.0, None, AluOpType.add)
                    else:
                        nc.scalar.copy(
                            s["ot"][:, s["o"] * CH:(s["o"] + 1) * CH], po[:])
                    if s["o"] == cpg - 1:
                        nc.sync.dma_start(
                            out=d_out[:, s["g"] * stage_cols:(s["g"] + 1) * stage_cols],
                            in_=s["ot"][:])
                    del st[j]
    nc.finalize()
    return nc


def _host_prep(coords, w0, w1, w2, w3, w4, w_last, rows, bf16=False, f16=False,
               enc5=False):
    """Split full inputs into per-core in_maps."""
    if f16:
        wdt = np.float16
    elif bf16:
        import ml_dtypes
        wdt = ml_dtypes.bfloat16
    else:
        wdt = np.float32
    coords = np.asarray(coords, np.float32)
    if enc5:
        smat = np.zeros((5, PE_SC), np.float16)
        for p in range(PE_SC - 2):
            k, f, s = p >> 2, (p >> 1) & 1, p & 1
            smat[f, p] = np.float16(2.0 ** (k - 1))
            smat[2 + f, p] = np.float16(2.0 ** (k - 1 - 6))
            smat[4, p] = 0.25 if s else 0.0
        smat[0, PE_SC - 2] = np.float16(COORD_S)
        smat[1, PE_SC - 1] = np.float16(COORD_S)
    else:
        smat = np.zeros((3, PE_SC), np.float32)
        for p in range(PE_SC - 2):
            k, f, s = p >> 2, (p >> 1) & 1, p & 1
            smat[f, p] = float(2.0 ** (k - 1))
            smat[2, p] = 0.25 if s else 0.0
        smat[0, PE_SC - 2] = COORD_S
        smat[1, PE_SC - 1] = COORD_S
    w0 = np.asarray(w0, np.float32)[0]              # [54, 256]
    w0s = np.empty((PE_SC, H), np.float32)
    w0s[:PE_SC - 2] = w0[2:]
    w0s[PE_SC - 2:] = w0[0:2] / np.float32(2.0 * np.pi * COORD_S)
    w0s = w0s.astype(wdt)
    wlT = np.ascontiguousarray(np.asarray(w_last, np.float32).T).astype(wdt)
    wmid_full = {1: np.asarray(w1, np.float32).astype(wdt),
                 2: np.asarray(w2, np.float32).astype(wdt),
                 3: np.asarray(w3, np.float32).astype(wdt),
                 4: np.asarray(w4, np.float32).astype(wdt)}
    ntile = {l: max(rows // TILE_ROWS[l], 1) for l in (1, 2, 3, 4)}
    in_maps = []
    for c in range(NCORES):
        sl = coords[c * rows:(c + 1) * rows]
        if enc5:
            chT = sl.T.astype(np.float16)                    # [2, rows] high
            clT = ((sl.T - chT.astype(np.float32)) * 64.0).astype(np.float16)
            c5 = np.empty((5, rows), np.float16)
            c5[0:2] = chT
            c5[2:4] = clT
            c5[4] = 1.0
            m = {"coords5": c5, "smat5": smat, "w0s": w0s, "wlT": wlT}
        else:
            ct3 = np.empty((3, rows), np.float32)
            ct3[0:2] = sl.T
            ct3[2] = 1.0
            m = {"coordsT3": ct3, "smat": smat, "w0s": w0s, "wlT": wlT}
        for l in (1, 2, 3, 4):
            w = wmid_full[l]
            t0 = c * rows // (N // w.shape[0]) if w.shape[0] * rows >= N else 0
            t0 = (c * rows) // (N // w.shape[0])
            m[f"w{l}"] = np.ascontiguousarray(w[t0:t0 + ntile[l]])
        in_maps.append(m)
    return in_maps


_BUILT = {}


def kernel(coords, w0, b0, w1, b1, w2, b2, w3, b3, w4, b4, w_last, b_last,
           f32r=True, lrelu_eng=None, variant="v3", G=3, bf16=False,
           f16=False, lazy_w=True, enc_mod=False, rotate=False):
    if variant == "v3":
        if rotate:
            le = lrelu_eng or ("hwp", "hwp", ("v2", "hwp", "hwp"), "hwp",
                              ("v2", "v2", "hwp"))
        else:
            le = lrelu_eng or ("hwp", "hwp", ("hwp", "hwp", "v2"), "hwp",
                              ("v2", "v2", "hwp"))
        key = ("v3", ROWS, repr(le), G, rotate)
        if key not in _BUILT:
            _BUILT[key] = _build3(ROWS, G=G, lrelu_eng=le, rotate=rotate)
        nc = _BUILT[key]
        in_maps = _host_prep(coords, w0, w1, w2, w3, w4, w_last, ROWS,
                             f16=True, enc5=True)
        res = run_bass_kernel_spmd(nc, in_maps, list(range(NCORES)), trace=TRACE)
        LAST["res"] = res
        out = np.empty((N, 3), np.float32)
        for c in range(NCORES):
            out[c * ROWS:(c + 1) * ROWS, :] = res.results[c]["out"].T
        return out
    if variant == "g3":
        le = lrelu_eng or ("hwp", "hwp", "v", "hwp", "hwp")
        key = ("g3", ROWS, bool(f32r), tuple(le), G, bf16, f16, lazy_w, enc_mod)
        if key not in _BUILT:
            _BUILT[key] = _build2(ROWS, G=G, f32r=f32r, lrelu_eng=le, bf16=bf16,
                                  f16=f16, lazy_w=lazy_w, enc_mod=enc_mod)
    else:
        bf16 = False
        f16 = False
        le = lrelu_eng or ("a", "a", "a", "a", "a")
        key = (ROWS, bool(f32r), tuple(le))
        if key not in _BUILT:
            _BUILT[key] = _build(ROWS, f32r=f32r, lrelu_eng=le)
    nc = _BUILT[key]
    in_maps = _host_prep(coords, w0, w1, w2, w3, w4, w_last, ROWS, bf16=bf16, f16=f16)
    res = run_bass_kernel_spmd(nc, in_maps, list(range(NCORES)), trace=TRACE)
    LAST["res"] = res
    out = np.empty((N, 3), np.float32)
    for c in range(NCORES):
        out[c * ROWS:(c + 1) * ROWS, :] = res.results[c]["out"].T
    return out



# revision 29
# speedup vs baseline: 1.1086x; 1.0241x over previous
"""Trainium2 Bass kernel for the LoE tiled-MLP (NeRF-style coordinate net).

Sharding: data-parallel over the pixel axis. N=262144 rows are split
contiguously across 8 cores (32768 rows each). Because the per-layer
expert tiles are contiguous row blocks, each core only ever needs a
contiguous slice of every weight tensor -> zero cross-core traffic.

Default build (_build3): all weights/activations fp16 (rel err ~8e-4,
budget 2e-2), chunks of 512 pixels processed in interleave groups of 3.
  - Positional encoding: coords pre-split on host into two f16 parts
    (xh=f16(x), xl'=f16((x-xh)*2^6)) so tps = smat5.T @ [xh,yh,xl',yl',1]
    is a single-pass f16 matmul reconstructing x to ~2^-24 rel; the
    smat5 entries are exact powers of two in f16. Then the magic-round
    r = t - round(t) on DVE and sin(2*pi*r) on ACT (table range +-pi).
  - The enc phase for group g+1 is emitted right after group g's L0 so
    the DVE/ACT enc chain clears before the PE needs it (no boundary
    stall), and mid-layer weight DMAs are emitted lazily a few groups
    ahead so the coords DMA is never queued behind megabytes of weights.
  - LeakyReLU(0.2) is a single HW Prelu op on ACT (the trig_and_small
    table holds sin+parametric_relu together, so one table load) for
    L0..L3, and a 2-op DVE form for L4 (PSUM->f16 copy + f16 max) to
    keep the ACT queue empty at the group boundary.
  - fp16 matmuls stream 1 row/cycle and keep the compiler's fast-weight-
    load path enabled (fp32-HIGH matmuls would disable it), so LDWEIGHTS
    hides under the previous matmul.
PSUM budget: 3 x [128,1024] layer tiles (6 banks) + 2 x 1-bank ring for
enc output / final [3,512] output = 8 banks exactly.
"""

import os
import sys

import numpy as np

sys.path.insert(0, "/opt/trn_rl_repo")

import concourse.bass as bass
import concourse.bacc as bacc
import concourse.mybir as mybir
import concourse.tile as tile
from concourse.alu_op_type import AluOpType
from concourse.bass_utils import run_bass_kernel_spmd

F32 = mybir.dt.float32
F32R = mybir.dt.float32r
BF16 = mybir.dt.bfloat16
F16 = mybir.dt.float16
ACT_SIN = mybir.ActivationFunctionType.Sin

N = 262144
NCORES = 8
ROWS = N // NCORES          # 32768 rows per core
CH = 512                    # pixels per chunk (psum free-dim, fp32 max)
K = 13                      # frequencies
H = 256
PE_SC = 2 * 2 * K + 2       # 52 sin/cos + 2 linearized coord rows
COORD_S = float(2.0 ** -11)  # tiny freq: sin(2*pi*s*c) ~ 2*pi*s*c, rel err 1.6e-6
MAGIC = float(1.5 * 2 ** 23)
TWO_PI = float(2.0 * np.pi)

# local (per-core) expert-tile row extents for layers 1..4
TILE_ROWS = {1: 65536, 2: 16384, 3: 4096, 4: 1024}

TRACE = False
LAST = {}


def _build(rows, f32r=True, stage_cols=2048, lrelu_eng=("a", "a", "a", "a", "a")):
    """Build the SPMD single-core Bass program for `rows` pixels."""
    nchunks = rows // CH
    stage_cols = min(stage_cols, rows)
    cpg = stage_cols // CH                       # chunks per DMA stage
    ntile = {l: max(rows // TILE_ROWS[l], 1) for l in (1, 2, 3, 4)}
    # chunk j -> local tile index for layer l
    tidx = {l: [min(j * CH // TILE_ROWS[l], ntile[l] - 1) for j in range(nchunks)]
            for l in (1, 2, 3, 4)}

    MDT = F32R if f32r else F32
    nc = bacc.Bacc()
    d_coords = nc.dram_tensor("coordsT3", [3, rows], F32, kind="ExternalInput")
    d_smat = nc.dram_tensor("smat", [3, PE_SC], F32, kind="ExternalInput")
    d_w0s = nc.dram_tensor("w0s", [PE_SC, H], MDT, kind="ExternalInput")
    d_wmid = {l: nc.dram_tensor(f"w{l}", [ntile[l], H, H], MDT, kind="ExternalInput")
              for l in (1, 2, 3, 4)}
    d_wl = nc.dram_tensor("wlT", [H, 3], MDT, kind="ExternalInput")
    d_out = nc.dram_tensor("out", [3, rows], F32, kind="ExternalOutput")

    def mdt(ap):
        return ap

    def lrelu(mode, xt, ps, rt):
        """xt(sbuf) = LeakyReLU_0.2(ps).  rt: scratch sbuf tile.

        Only ACT and DVE can read PSUM, and at most one tensor operand of a
        DVE op may live in PSUM, hence the two-pass forms.
        """
        if mode == "a":      # ACT relu + DVE combine
            nc.scalar.activation(rt[:], ps[:], mybir.ActivationFunctionType.Relu,
                                 scale=0.8)
            nc.vector.scalar_tensor_tensor(xt[:], ps[:], 0.2, rt[:],
                                           AluOpType.mult, AluOpType.add)
        elif mode == "v":    # DVE relu + DVE combine
            nc.vector.tensor_scalar(rt[:], ps[:], 0.0, 0.8,
                                    AluOpType.max, AluOpType.mult)
            nc.vector.scalar_tensor_tensor(xt[:], ps[:], 0.2, rt[:],
                                           AluOpType.mult, AluOpType.add)
        elif mode == "hwl":  # single ACT op, HW Lrelu table (alpha slope)
            nc.scalar.activation(xt[:], ps[:], mybir.ActivationFunctionType.Lrelu,
                                 alpha=0.2)
        elif mode == "hwp":  # single ACT op, HW Prelu table (alpha slope)
            nc.scalar.activation(xt[:], ps[:], mybir.ActivationFunctionType.Prelu,
                                 alpha=0.2)
        else:
            raise ValueError(mode)

    with tile.TileContext(nc) as tc:
        with (
            tc.tile_pool(name="wp", bufs=1) as wp,
            tc.tile_pool(name="io", bufs=2) as iop,
            tc.tile_pool(name="ac", bufs=2) as acp,
            tc.tile_pool(name="psa", bufs=2, space="PSUM") as ppa,
            tc.tile_pool(name="psb", bufs=3, space="PSUM") as ppb,
        ):
            # ---- resident weights (DMA once, first-use order) ----
            smat_sb = wp.tile([3, PE_SC], F32, tag="smat")
            nc.sync.dma_start(out=smat_sb[:], in_=d_smat[:])
            w0s_sb = wp.tile([PE_SC, H], MDT, tag="w0s")
            nc.sync.dma_start(out=w0s_sb[:], in_=d_w0s[:])
            wl_sb = []
            for kb in range(2):
                t = wp.tile([128, 3], MDT, tag=f"wl{kb}")
                nc.sync.dma_start(out=t[:], in_=d_wl[kb * 128:(kb + 1) * 128, :])
                wl_sb.append(t)

            wmid_sb = {l: [[None, None] for _ in range(ntile[l])] for l in (1, 2, 3, 4)}
            order = []
            for l in (1, 2, 3, 4):
                for t in range(ntile[l]):
                    first = min(j for j in range(nchunks) if tidx[l][j] == t)
                    order.append((first, l, t))
            order.sort()
            for _, l, t in order:
                for kb in range(2):
                    w = wp.tile([128, H], MDT, tag=f"w{l}_{t}_{kb}")
                    nc.sync.dma_start(
                        out=w[:], in_=d_wmid[l][t, kb * 128:(kb + 1) * 128, :])
                    wmid_sb[l][t][kb] = w

            # ---- main chunk loop: pairs of chunks, layer-interleaved ----
            # PE is an in-order queue: emitting chunk j+1's matmuls right
            # after chunk j's same-layer matmuls means every PE wait (on the
            # LeakyReLU chain) has independent work queued behind it.
            cr = None
            ot = None
            st = {}
            for jj in range(0, nchunks, 2):
                pair = [j for j in (jj, jj + 1) if j < nchunks]
                for j in pair:
                    g, o = divmod(j, cpg)
                    if o == 0:
                        cr = iop.tile([3, stage_cols], F32, tag="cr")
                        nc.sync.dma_start(
                            out=cr[:],
                            in_=d_coords[:, g * stage_cols:(g + 1) * stage_cols])
                        ot = iop.tile([3, stage_cols], F32, tag="ot")
                    rc = cr[:, o * CH:(o + 1) * CH]      # [3, 512] coords+ones
                    tps = ppa.tile([PE_SC, CH], F32, tag="ang")
                    nc.tensor.matmul(tps[:], smat_sb[:], rc, start=True, stop=True)
                    st[j] = {"rc": rc, "tps": tps, "ot": ot, "g": g, "o": o}
                for j in pair:
                    s = st[j]
                    rnd = acp.tile([PE_SC, CH], F32, tag="rnd")
                    nc.vector.tensor_scalar(rnd[:], s["tps"][:], MAGIC, MAGIC,
                                            AluOpType.add, AluOpType.subtract)
                    frac = acp.tile([PE_SC, CH], F32, tag="frac")
                    nc.vector.tensor_tensor(frac[:], s["tps"][:], rnd[:],
                                            AluOpType.subtract)
                    sc = acp.tile([PE_SC, CH], MDT, tag="sc")
                    nc.scalar.activation(sc[:], frac[:], ACT_SIN, scale=TWO_PI)
                    s["sc"] = sc
                for j in pair:
                    s = st[j]
                    ps = ppb.tile([128, 2 * CH], F32, tag="lps")
                    for ob in range(2):
                        nc.tensor.matmul(ps[:, ob * CH:(ob + 1) * CH],
                                         w0s_sb[:, ob * 128:(ob + 1) * 128],
                                         s["sc"][:], start=True, stop=True)
                    x = acp.tile([128, 2 * CH], MDT, tag="x0")
                    rt = acp.tile([128, 2 * CH], F32, tag="rt")
                    lrelu(lrelu_eng[0], x, ps, rt)
                    s["x"] = x
                for l in (1, 2, 3, 4):
                    for j in pair:
                        s = st[j]
                        wt = wmid_sb[l][tidx[l][j]]
                        ps = ppb.tile([128, 2 * CH], F32, tag="lps")
                        for ob in range(2):
                            osl = slice(ob * CH, (ob + 1) * CH)
                            wsl = slice(ob * 128, (ob + 1) * 128)
                            for kb in range(2):
                                nc.tensor.matmul(
                                    ps[:, osl], wt[kb][:, wsl],
                                    s["x"][:, kb * CH:(kb + 1) * CH],
                                    start=(kb == 0), stop=(kb == 1))
                        xn = acp.tile([128, 2 * CH], MDT, tag=f"x{l}")
                        rt = acp.tile([128, 2 * CH], F32, tag="rt")
                        lrelu(lrelu_eng[l], xn, ps, rt)
                        s["x"] = xn
                for j in pair:
                    s = st[j]
                    po = ppb.tile([3, CH], F32, tag="lps")
                    for kb in range(2):
                        nc.tensor.matmul(po[:], wl_sb[kb][:],
                                         s["x"][:, kb * CH:(kb + 1) * CH],
                                         start=(kb == 0), stop=(kb == 1))
                    nc.scalar.copy(s["ot"][:, s["o"] * CH:(s["o"] + 1) * CH], po[:])
                    if gi == len(groups) - 1:
                        nc.sync.dma_start(
                            out=d_out[:, j * CH:(j + 1) * CH],
                            in_=s["ot"][:, s["o"] * CH:(s["o"] + 1) * CH])
                    elif s["o"] == cpg - 1:
                        nc.sync.dma_start(
                            out=d_out[:, s["g"] * stage_cols:(s["g"] + 1) * stage_cols],
                            in_=s["ot"][:])
                    del st[j]
    nc.finalize()
    return nc


def _build2(rows, G=3, lrelu_eng=("hwp", "hwp", "v", "hwp", "hwp"),
            stage_cols=2048, f32r=True, bf16=False, f16=False, lazy_w=True,
            enc_mod=False, prefetch_groups=3, copy_eng="v"):
    """Restructured SPMD program: G-chunk interleave groups.

    Per group of G chunks, each layer emits the G chunks' matmuls
    back-to-back so the PE always has >=2 chunks of independent work
    queued behind every LeakyReLU-chain wait. LeakyReLU is a single
    ACT-engine Prelu op ("hwp") for most layers; one layer runs on DVE
    ("v", two ops) to balance engine load. Output copy PSUM->SBUF is on
    DVE. PSUM budget: lps ring 3x2 banks + ang/po ring 2x1 bank = 8.

    bf16: weights+activations in bf16 (enc path stays fp32). Halves
    LDWEIGHTS time and weight DMA.
    lazy_w: emit mid-layer weight DMAs just-in-time (prefetch_groups
    ahead) so the stage-0 coords DMA isn't queued behind 11MB of
    weights on the in-order sync queue.
    enc_mod: one-op range reduction frac2 = (t mod 1) - 0.5 on DVE and
    sin(-2*pi*frac2) = sin(2*pi*t) on ACT, instead of the two-op
    magic-round form.
    """
    nchunks = rows // CH
    stage_cols = min(stage_cols, rows)
    cpg = stage_cols // CH
    ntile = {l: max(rows // TILE_ROWS[l], 1) for l in (1, 2, 3, 4)}
    tidx = {l: [min(j * CH // TILE_ROWS[l], ntile[l] - 1) for j in range(nchunks)]
            for l in (1, 2, 3, 4)}

    MDT = F16 if f16 else (BF16 if bf16 else (F32R if f32r else F32))
    nc = bacc.Bacc()
    d_coords = nc.dram_tensor("coordsT3", [3, rows], F32, kind="ExternalInput")
    d_smat = nc.dram_tensor("smat", [3, PE_SC], F32, kind="ExternalInput")
    d_w0s = nc.dram_tensor("w0s", [PE_SC, H], MDT, kind="ExternalInput")
    d_wmid = {l: nc.dram_tensor(f"w{l}", [ntile[l], H, H], MDT, kind="ExternalInput")
              for l in (1, 2, 3, 4)}
    d_wl = nc.dram_tensor("wlT", [H, 3], MDT, kind="ExternalInput")
    d_out = nc.dram_tensor("out", [3, rows], F32, kind="ExternalOutput")

    def lrelu(mode, xt, ps, scratch_pool):
        if mode == "hwp":     # single ACT op, HW Prelu table (alpha slope)
            nc.scalar.activation(xt[:], ps[:], mybir.ActivationFunctionType.Prelu,
                                 alpha=0.2)
        elif mode == "a":     # ACT relu + DVE combine
            rt = scratch_pool.tile(list(xt.shape), F32, tag="rt")
            nc.scalar.activation(rt[:], ps[:], mybir.ActivationFunctionType.Relu,
                                 scale=0.8)
            nc.vector.scalar_tensor_tensor(xt[:], ps[:], 0.2, rt[:],
                                           AluOpType.mult, AluOpType.add)
        elif mode == "v":     # DVE-only: relu+scale then combine
            rt = scratch_pool.tile(list(xt.shape), F32, tag="rt")
            nc.vector.tensor_scalar(rt[:], ps[:], 0.0, 0.8,
                                    AluOpType.max, AluOpType.mult)
            nc.vector.scalar_tensor_tensor(xt[:], ps[:], 0.2, rt[:],
                                           AluOpType.mult, AluOpType.add)
        else:
            raise ValueError(mode)

    with tile.TileContext(nc) as tc:
        with (
            tc.tile_pool(name="wp", bufs=1) as wp,
            tc.tile_pool(name="io", bufs=2) as iop,
            tc.tile_pool(name="ac", bufs=3) as acp,
            tc.tile_pool(name="rt", bufs=2) as rtp,
            tc.tile_pool(name="psa", bufs=2, space="PSUM") as ppa,
            tc.tile_pool(name="psb", bufs=3, space="PSUM") as ppb,
        ):
            # ---- small resident weights up front ----
            smat_sb = wp.tile([3, PE_SC], F32, tag="smat")
            nc.sync.dma_start(out=smat_sb[:], in_=d_smat[:])
            w0s_sb = wp.tile([PE_SC, H], MDT, tag="w0s")
            nc.sync.dma_start(out=w0s_sb[:], in_=d_w0s[:])
            wl_sb = []
            for kb in range(2):
                t = wp.tile([128, 3], MDT, tag=f"wl{kb}")
                nc.sync.dma_start(out=t[:], in_=d_wl[kb * 128:(kb + 1) * 128, :])
                wl_sb.append(t)

            # ---- mid-layer weights: eager (first-use order) or lazy ----
            wmid_sb = {l: [None] * ntile[l] for l in (1, 2, 3, 4)}

            def ensure_w(l, t):
                if wmid_sb[l][t] is None:
                    blk = []
                    for kb in range(2):
                        w = wp.tile([128, H], MDT, tag=f"w{l}_{t}_{kb}")
                        nc.sync.dma_start(
                            out=w[:], in_=d_wmid[l][t, kb * 128:(kb + 1) * 128, :])
                        blk.append(w)
                    wmid_sb[l][t] = blk

            if not lazy_w:
                order = []
                for l in (1, 2, 3, 4):
                    for t in range(ntile[l]):
                        first = min(j for j in range(nchunks) if tidx[l][j] == t)
                        order.append((first, l, t))
                order.sort()
                for _, l, t in order:
                    ensure_w(l, t)

            # ---- main loop: groups of G chunks, layer-lockstep ----
            cr = None
            ot = None
            st = {}
            for gg in range(0, nchunks, G):
                group = list(range(gg, min(gg + G, nchunks)))
                for j in group:
                    g, o = divmod(j, cpg)
                    if o == 0:
                        cr = iop.tile([3, stage_cols], F32, tag="cr")
                        nc.sync.dma_start(
                            out=cr[:],
                            in_=d_coords[:, g * stage_cols:(g + 1) * stage_cols])
                        ot = iop.tile([3, stage_cols], F32, tag="ot")
                    rc = cr[:, o * CH:(o + 1) * CH]
                    tps = ppa.tile([PE_SC, CH], F32, tag="ang")
                    nc.tensor.matmul(tps[:], smat_sb[:], rc, start=True, stop=True)
                    st[j] = {"tps": tps, "ot": ot, "g": g, "o": o}
                if lazy_w:
                    lim = min(gg + G * prefetch_groups, nchunks)
                    for l in (1, 2, 3, 4):
                        for j in range(gg, lim):
                            ensure_w(l, tidx[l][j])
                for j in group:
                    s = st[j]
                    if enc_mod:
                        frac = acp.tile([PE_SC, CH], F32, tag="frac")
                        nc.vector.tensor_scalar(frac[:], s["tps"][:], 1.0, 0.5,
                                                AluOpType.mod, AluOpType.subtract)
                        sc = acp.tile([PE_SC, CH], MDT, tag="sc")
                        nc.scalar.activation(sc[:], frac[:], ACT_SIN, scale=-TWO_PI)
                    else:
                        rnd = acp.tile([PE_SC, CH], F32, tag="rnd")
                        nc.vector.tensor_scalar(rnd[:], s["tps"][:], MAGIC, MAGIC,
                                                AluOpType.add, AluOpType.subtract)
                        frac = acp.tile([PE_SC, CH], F32, tag="frac")
                        nc.vector.tensor_tensor(frac[:], s["tps"][:], rnd[:],
                                                AluOpType.subtract)
                        sc = acp.tile([PE_SC, CH], MDT, tag="sc")
                        nc.scalar.activation(sc[:], frac[:], ACT_SIN, scale=TWO_PI)
                    s["sc"] = sc
                for pos, j in enumerate(group):
                    s = st[j]
                    ps = ppb.tile([128, 2 * CH], F32, tag="lps")
                    for ob in range(2):
                        nc.tensor.matmul(ps[:, ob * CH:(ob + 1) * CH],
                                         w0s_sb[:, ob * 128:(ob + 1) * 128],
                                         s["sc"][:], start=True, stop=True)
                    x = acp.tile([128, 2 * CH], MDT, tag="x0")
                    lrelu(lmode(0, pos), x, ps, rtp)
                    s["x"] = x
                for l in (1, 2, 3, 4):
                    for j in group:
                        s = st[j]
                        wt = wmid_sb[l][tidx[l][j]]
                        ps = ppb.tile([128, 2 * CH], F32, tag="lps")
                        for ob in range(2):
                            osl = slice(ob * CH, (ob + 1) * CH)
                            wsl = slice(ob * 128, (ob + 1) * 128)
                            for kb in range(2):
                                nc.tensor.matmul(
                                    ps[:, osl], wt[kb][:, wsl],
                                    s["x"][:, kb * CH:(kb + 1) * CH],
                                    start=(kb == 0), stop=(kb == 1))
                        xn = acp.tile([128, 2 * CH], MDT, tag=f"x{l}")
                        lrelu(lrelu_eng[l], xn, ps, rtp)
                        s["x"] = xn
                for j in group:
                    s = st[j]
                    po = ppa.tile([3, CH], F32, tag="ang")
                    for kb in range(2):
                        nc.tensor.matmul(po[:], wl_sb[kb][:],
                                         s["x"][:, kb * CH:(kb + 1) * CH],
                                         start=(kb == 0), stop=(kb == 1))
                    if copy_eng == "v":
                        nc.vector.tensor_scalar(
                            s["ot"][:, s["o"] * CH:(s["o"] + 1) * CH], po[:],
                            0.0, None, AluOpType.add)
                    else:
                        nc.scalar.copy(
                            s["ot"][:, s["o"] * CH:(s["o"] + 1) * CH], po[:])
                    if gi == len(groups) - 1:
                        nc.sync.dma_start(
                            out=d_out[:, j * CH:(j + 1) * CH],
                            in_=s["ot"][:, s["o"] * CH:(s["o"] + 1) * CH])
                    elif s["o"] == cpg - 1:
                        nc.sync.dma_start(
                            out=d_out[:, s["g"] * stage_cols:(s["g"] + 1) * stage_cols],
                            in_=s["ot"][:])
                    del st[j]
    nc.finalize()
    return nc


def _build3(rows, G=3,
            lrelu_eng=("hwp", "hwp", ("hwp", "hwp", "v2"), "hwp",
                       ("v2", "v2", "hwp")),
            stage_cols=2048, copy_eng="v", prefetch_groups=3, rotate=False):
    """f16 build with single-pass f16 positional-encoding matmul and a
    software-pipelined encoding stage.

    Coordinates are pre-split on host into two f16 parts
    (xh = f16(x), xl' = f16((x - xh) * 2^6)) so the enc matmul
      tps = smat5.T @ [xh, yh, xl', yl', 1]
    is pure f16 (1 cycle/row, keeps FWL enabled) while reconstructing
    x to ~2^-24 relative: smat5 rows hold 2^(k-1) and 2^(k-1-6), all
    exact powers of two in f16. The enc phase for group g+1 is emitted
    right after group g's L0 so the rnd/frac/sin chain clears long
    before the PE needs sc, killing the group-boundary stall.
    """
    nchunks = rows // CH
    stage_cols = min(stage_cols, rows)
    cpg = stage_cols // CH
    ntile = {l: max(rows // TILE_ROWS[l], 1) for l in (1, 2, 3, 4)}
    tidx = {l: [min(j * CH // TILE_ROWS[l], ntile[l] - 1) for j in range(nchunks)]
            for l in (1, 2, 3, 4)}

    MDT = F16
    nc = bacc.Bacc()
    d_coords = nc.dram_tensor("coords5", [5, rows], F16, kind="ExternalInput")
    d_smat = nc.dram_tensor("smat5", [5, PE_SC], F16, kind="ExternalInput")
    d_w0s = nc.dram_tensor("w0s", [PE_SC, H], MDT, kind="ExternalInput")
    d_wmid = {l: nc.dram_tensor(f"w{l}", [ntile[l], H, H], MDT, kind="ExternalInput")
              for l in (1, 2, 3, 4)}
    d_wl = nc.dram_tensor("wlT", [H, 3], MDT, kind="ExternalInput")
    d_out = nc.dram_tensor("out", [3, rows], F32, kind="ExternalOutput")

    def lrelu(mode, xt, ps, rtp):
        if mode == "hwp":
            nc.scalar.activation(xt[:], ps[:], mybir.ActivationFunctionType.Prelu,
                                 alpha=0.2)
        elif mode == "a":
            rt = rtp.tile(list(xt.shape), F32, tag="rt")
            nc.scalar.activation(rt[:], ps[:], mybir.ActivationFunctionType.Relu,
                                 scale=0.8)
            nc.vector.scalar_tensor_tensor(xt[:], ps[:], 0.2, rt[:],
                                           AluOpType.mult, AluOpType.add)
        elif mode == "v":
            rt = rtp.tile(list(xt.shape), F32, tag="rt")
            nc.vector.tensor_scalar(rt[:], ps[:], 0.0, 0.8,
                                    AluOpType.max, AluOpType.mult)
            nc.vector.scalar_tensor_tensor(xt[:], ps[:], 0.2, rt[:],
                                           AluOpType.mult, AluOpType.add)
        elif mode == "hwp2":  # Prelu in two halves: first half ready sooner
            h = xt.shape[-1] // 2
            nc.scalar.activation(xt[:, :h], ps[:, :h],
                                 mybir.ActivationFunctionType.Prelu, alpha=0.2)
            nc.scalar.activation(xt[:, h:], ps[:, h:],
                                 mybir.ActivationFunctionType.Prelu, alpha=0.2)
        elif mode == "v2":    # DVE: PSUM->f16 copy, then all-SBUF f16 max
            yv = rtp.tile(list(xt.shape), F16, tag="rt16")
            nc.vector.tensor_scalar(yv[:], ps[:], 0.0, None, AluOpType.add)
            nc.vector.scalar_tensor_tensor(xt[:], yv[:], 0.2, yv[:],
                                           AluOpType.mult, AluOpType.max)
        elif mode == "v2h":   # v2 in column halves: first half ready sooner,
            h = xt.shape[-1] // 2  # so the next layer's kb0 matmuls can start
            yv = rtp.tile(list(xt.shape), F16, tag="rt16")
            for sl in (slice(0, h), slice(h, 2 * h)):
                nc.vector.tensor_scalar(yv[:, sl], ps[:, sl], 0.0, None,
                                        AluOpType.add)
                nc.vector.scalar_tensor_tensor(xt[:, sl], yv[:, sl], 0.2,
                                               yv[:, sl], AluOpType.mult,
                                               AluOpType.max)
        else:
            raise ValueError(mode)

    with tile.TileContext(nc) as tc:
        with (
            tc.tile_pool(name="wp", bufs=1) as wp,
            tc.tile_pool(name="io", bufs=2) as iop,
            tc.tile_pool(name="ac", bufs=3) as acp,
            tc.tile_pool(name="rt", bufs=2) as rtp,
            tc.tile_pool(name="psa", bufs=2, space="PSUM") as ppa,
            tc.tile_pool(name="psb", bufs=3, space="PSUM") as ppb,
        ):
            smat_sb = wp.tile([5, PE_SC], F16, tag="smat")
            nc.sync.dma_start(out=smat_sb[:], in_=d_smat[:])

            wmid_sb = {l: [None] * ntile[l] for l in (1, 2, 3, 4)}

            def ensure_w(l, t):
                if wmid_sb[l][t] is None:
                    blk = []
                    for kb in range(2):
                        w = wp.tile([128, H], MDT, tag=f"w{l}_{t}_{kb}")
                        nc.sync.dma_start(
                            out=w[:], in_=d_wmid[l][t, kb * 128:(kb + 1) * 128, :])
                        blk.append(w)
                    wmid_sb[l][t] = blk

            st = {}
            io_state = {"cr": None, "ot": None}

            def emit_enc(group):
                """Enc phase for `group`: matmul + rnd/frac (DVE) + sin (ACT)."""
                for j in group:
                    g, o = divmod(j, cpg)
                    if o == 0:
                        if g in io_state:
                            io_state["cr"], io_state["ot"] = io_state.pop(g)
                        else:
                            cr_t = iop.tile([5, stage_cols], F16, tag="cr")
                            nc.sync.dma_start(
                                out=cr_t[:],
                                in_=d_coords[:,
                                             g * stage_cols:(g + 1) * stage_cols])
                            ot_t = iop.tile([3, stage_cols], F32, tag="ot")
                            io_state["cr"], io_state["ot"] = cr_t, ot_t
                    rc = io_state["cr"][:, o * CH:(o + 1) * CH]
                    tps = ppa.tile([PE_SC, CH], F32, tag="ang")
                    nc.tensor.matmul(tps[:], smat_sb[:], rc, start=True, stop=True)
                    st[j] = {"tps": tps, "ot": io_state["ot"], "g": g, "o": o}
                for j in group:
                    s = st[j]
                    rnd = acp.tile([PE_SC, CH], F32, tag="rnd")
                    nc.vector.tensor_scalar(rnd[:], s["tps"][:], MAGIC, MAGIC,
                                            AluOpType.add, AluOpType.subtract)
                    frac = acp.tile([PE_SC, CH], F32, tag="frac")
                    nc.vector.tensor_tensor(frac[:], s["tps"][:], rnd[:],
                                            AluOpType.subtract)
                    sc = acp.tile([PE_SC, CH], MDT, tag="sc")
                    nc.scalar.activation(sc[:], frac[:], ACT_SIN, scale=TWO_PI)
                    s["sc"] = sc

            def lmode(l, pos):
                e = lrelu_eng[l]
                return e if isinstance(e, str) else e[min(pos, len(e) - 1)]

            groups = [list(range(gg, min(gg + G, nchunks)))
                      for gg in range(0, nchunks, G)]
            if len(groups) > 1 and len(groups[-1]) < G:
                groups[-2].extend(groups.pop())
            emit_enc(groups[0])
            if nchunks > cpg:
                cr_p = iop.tile([5, stage_cols], F16, tag="cr")
                nc.sync.dma_start(out=cr_p[:],
                                  in_=d_coords[:, stage_cols:2 * stage_cols])
                ot_p = iop.tile([3, stage_cols], F32, tag="ot")
                io_state[1] = (cr_p, ot_p)
            w0s_sb = wp.tile([PE_SC, H], MDT, tag="w0s")
            nc.sync.dma_start(out=w0s_sb[:], in_=d_w0s[:])
            wl_sb = []
            for kb in range(2):
                t = wp.tile([128, 3], MDT, tag=f"wl{kb}")
                nc.sync.dma_start(out=t[:], in_=d_wl[kb * 128:(kb + 1) * 128, :])
                wl_sb.append(t)
            for l in (1, 2, 3, 4):
                for j in range(min(G * (prefetch_groups + 5), nchunks)):
                    ensure_w(l, tidx[l][j])
            for gi, group in enumerate(groups):
                # L0
                for pos, j in enumerate(group):
                    s = st[j]
                    ps = ppb.tile([128, 2 * CH], F32, tag="lps")
                    for ob in range(2):
                        nc.tensor.matmul(ps[:, ob * CH:(ob + 1) * CH],
                                         w0s_sb[:, ob * 128:(ob + 1) * 128],
                                         s["sc"][:], start=True, stop=True)
                    x = acp.tile([128, 2 * CH], MDT, tag="x0")
                    lrelu(lmode(0, pos), x, ps, rtp)
                    s["x"] = x
                # L1..L4; the enc phase for group g+1 is emitted after L2 so
                # the DVE is free during L1/L2 (mid-layer v2 units run promptly)
                # while sin(g+1) still clears an entire L3+L4+last ahead of its
                # consumer
                for l in (1, 2, 3, 4):
                    if l == 3 and gi + 1 < len(groups):
                        emit_enc(groups[gi + 1])
                        lim = min(groups[gi + 1][-1] + 1 + G * (prefetch_groups - 1),
                                  nchunks)
                        for lw in (1, 2, 3, 4):
                            for j in range(groups[gi + 1][0], lim):
                                ensure_w(lw, tidx[lw][j])
                    lorder = (group[1:] + group[:1]) if (rotate and l >= 3) \
                        else group
                    for pos, j in enumerate(lorder):
                        s = st[j]
                        wt = wmid_sb[l][tidx[l][j]]
                        ps = ppb.tile([128, 2 * CH], F32, tag="lps")
                        for ob in range(2):
                            osl = slice(ob * CH, (ob + 1) * CH)
                            wsl = slice(ob * 128, (ob + 1) * 128)
                            for kb in range(2):
                                nc.tensor.matmul(
                                    ps[:, osl], wt[kb][:, wsl],
                                    s["x"][:, kb * CH:(kb + 1) * CH],
                                    start=(kb == 0), stop=(kb == 1))
                        xn = acp.tile([128, 2 * CH], MDT, tag=f"x{l}")
                        lrelu(lmode(l, pos), xn, ps, rtp)
                        s["x"] = xn
                # last layer + output copy
                for pos, j in enumerate((group[1:] + group[:1]) if rotate
                                        else group):
                    s = st[j]
                    po = ppa.tile([3, CH], F32, tag="ang")
                    for kb in range(2):
                        nc.tensor.matmul(po[:], wl_sb[kb][:],
                                         s["x"][:, kb * CH:(kb + 1) * CH],
                                         start=(kb == 0), stop=(kb == 1))
                    if copy_eng == "v":
                        nc.vector.tensor_scalar(
                            s["ot"][:, s["o"] * CH:(s["o"] + 1) * CH], po[:],
                            0.0, None, AluOpType.add)
                    else:
                        nc.scalar.copy(
                            s["ot"][:, s["o"] * CH:(s["o"] + 1) * CH], po[:])
                    if gi == len(groups) - 1:
                        nc.sync.dma_start(
                            out=d_out[:, j * CH:(j + 1) * CH],
                            in_=s["ot"][:, s["o"] * CH:(s["o"] + 1) * CH])
                    elif s["o"] == cpg - 1:
                        nc.sync.dma_start(
                            out=d_out[:, s["g"] * stage_cols:(s["g"] + 1) * stage_cols],
                            in_=s["ot"][:])
                    del st[j]
    nc.finalize()
    return nc


def _host_prep(coords, w0, w1, w2, w3, w4, w_last, rows, bf16=False, f16=False,
               enc5=False):
    """Split full inputs into per-core in_maps."""
    if f16:
        wdt = np.float16
    elif bf16:
        import ml_dtypes
        wdt = ml_dtypes.bfloat16
    else:
        wdt = np.float32
    coords = np.asarray(coords, np.float32)
    if enc5:
        smat = np.zeros((5, PE_SC), np.float16)
        for p in range(PE_SC - 2):
            k, f, s = p >> 2, (p >> 1) & 1, p & 1
            smat[f, p] = np.float16(2.0 ** (k - 1))
            smat[2 + f, p] = np.float16(2.0 ** (k - 1 - 6))
            smat[4, p] = 0.25 if s else 0.0
        smat[0, PE_SC - 2] = np.float16(COORD_S)
        smat[1, PE_SC - 1] = np.float16(COORD_S)
    else:
        smat = np.zeros((3, PE_SC), np.float32)
        for p in range(PE_SC - 2):
            k, f, s = p >> 2, (p >> 1) & 1, p & 1
            smat[f, p] = float(2.0 ** (k - 1))
            smat[2, p] = 0.25 if s else 0.0
        smat[0, PE_SC - 2] = COORD_S
        smat[1, PE_SC - 1] = COORD_S
    w0 = np.asarray(w0, np.float32)[0]              # [54, 256]
    w0s = np.empty((PE_SC, H), np.float32)
    w0s[:PE_SC - 2] = w0[2:]
    w0s[PE_SC - 2:] = w0[0:2] / np.float32(2.0 * np.pi * COORD_S)
    w0s = w0s.astype(wdt)
    wlT = np.ascontiguousarray(np.asarray(w_last, np.float32).T).astype(wdt)
    wmid_full = {1: np.asarray(w1, np.float32).astype(wdt),
                 2: np.asarray(w2, np.float32).astype(wdt),
                 3: np.asarray(w3, np.float32).astype(wdt),
                 4: np.asarray(w4, np.float32).astype(wdt)}
    ntile = {l: max(rows // TILE_ROWS[l], 1) for l in (1, 2, 3, 4)}
    in_maps = []
    for c in range(NCORES):
        sl = coords[c * rows:(c + 1) * rows]
        if enc5:
            chT = sl.T.astype(np.float16)                    # [2, rows] high
            clT = ((sl.T - chT.astype(np.float32)) * 64.0).astype(np.float16)
            c5 = np.empty((5, rows), np.float16)
            c5[0:2] = chT
            c5[2:4] = clT
            c5[4] = 1.0
            m = {"coords5": c5, "smat5": smat, "w0s": w0s, "wlT": wlT}
        else:
            ct3 = np.empty((3, rows), np.float32)
            ct3[0:2] = sl.T
            ct3[2] = 1.0
            m = {"coordsT3": ct3, "smat": smat, "w0s": w0s, "wlT": wlT}
        for l in (1, 2, 3, 4):
            w = wmid_full[l]
            t0 = c * rows // (N // w.shape[0]) if w.shape[0] * rows >= N else 0
            t0 = (c * rows) // (N // w.shape[0])
            m[f"w{l}"] = np.ascontiguousarray(w[t0:t0 + ntile[l]])
        in_maps.append(m)
    return in_maps


_BUILT = {}


def kernel(coords, w0, b0, w1, b1, w2, b2, w3, b3, w4, b4, w_last, b_last,
           f32r=True, lrelu_eng=None, variant="v3", G=3, bf16=False,
           f16=False, lazy_w=True, enc_mod=False, rotate=False):
    if variant == "v3":
        if rotate:
            le = lrelu_eng or ("hwp", "hwp", ("v2", "hwp", "hwp"), "hwp",
                              ("v2", "v2", "hwp"))
        else:
            le = lrelu_eng or ("hwp", "hwp", ("hwp", "hwp", "v2"), "hwp",
                              ("v2", "v2", "hwp"))
        key = ("v3", ROWS, repr(le), G, rotate)
        if key not in _BUILT:
            _BUILT[key] = _build3(ROWS, G=G, lrelu_eng=le, rotate=rotate)
        nc = _BUILT[key]
        in_maps = _host_prep(coords, w0, w1, w2, w3, w4, w_last, ROWS,
                             f16=True, enc5=True)
        res = run_bass_kernel_spmd(nc, in_maps, list(range(NCORES)), trace=TRACE)
        LAST["res"] = res
        out = np.empty((N, 3), np.float32)
        for c in range(NCORES):
            out[c * ROWS:(c + 1) * ROWS, :] = res.results[c]["out"].T
        return out
    if variant == "g3":
        le = lrelu_eng or ("hwp", "hwp", "v", "hwp", "hwp")
        key = ("g3", ROWS, bool(f32r), tuple(le), G, bf16, f16, lazy_w, enc_mod)
        if key not in _BUILT:
            _BUILT[key] = _build2(ROWS, G=G, f32r=f32r, lrelu_eng=le, bf16=bf16,
                                  f16=f16, lazy_w=lazy_w, enc_mod=enc_mod)
    else:
        bf16 = False
        f16 = False
        le = lrelu_eng or ("a", "a", "a", "a", "a")
        key = (ROWS, bool(f32r), tuple(le))
        if key not in _BUILT:
            _BUILT[key] = _build(ROWS, f32r=f32r, lrelu_eng=le)
    nc = _BUILT[key]
    in_maps = _host_prep(coords, w0, w1, w2, w3, w4, w_last, ROWS, bf16=bf16, f16=f16)
    res = run_bass_kernel_spmd(nc, in_maps, list(range(NCORES)), trace=TRACE)
    LAST["res"] = res
    out = np.empty((N, 3), np.float32)
    for c in range(NCORES):
        out[c * ROWS:(c + 1) * ROWS, :] = res.results[c]["out"].T
    return out

